# revision 1
# baseline (speedup 1.0000x reference)
"""Multi-head self-attention (B=4, S=2048, D=1024, H=16, Hd=64) on 8 TRN2 cores.

Sharding: tensor-parallel over heads. Core c owns heads 2c, 2c+1:
  - computes Q^T/K^T [128hd, tok] (f32r) and V (bf16) for its 2 heads
  - flash-style attention per (batch, head): S^T = K^T.T @ Q^T into
    [128,1024] psum stripes, exp on ACT (scale=1/8 folded in), AV +
    denominator via ones-append (M=65), reciprocal_approx_fast +
    PE ones-broadcast, normalize on DVE -> A^T (bf16)
  - 8 pipelined AllGathers (one per batch x local-head row-block)
  - out-proj: out^T[:, c-slice] = wo_perm_c.T @ A'^T + bo_c (bf16 matmul,
    fp32 psum); host transposes/concats column slices.

Matmul dtype: float32r (~1.5e-4 rel err, full PE rate at N>=512) for
projections/scores; bf16 for the P/V/AV/out-proj path (psum always fp32).
"""
import numpy as np

B, S, D, H, HD = 4, 2048, 1024, 16, 64
N_CORES = 8
TOK = B * S            # 8192
HPC = H // N_CORES     # 2 heads per core
CW = HPC * HD          # 128 cols per core
QS = 1024              # query stripe
NKT = S // 128         # 16 kt chunks per batch
NQS = S // QS          # 2 q stripes per batch
NTB = TOK // 512       # 16 token blocks overall

_CACHE = {}


def _build():
    import concourse.bacc as bacc
    import concourse.mybir as mybir
    import concourse.tile as tile

    F32 = mybir.dt.float32
    F32R = mybir.dt.float32r
    BF16 = mybir.dt.bfloat16
    AF = mybir.ActivationFunctionType

    nc = bacc.Bacc(trn_type="TRN2", target_bir_lowering=False, debug=False,
                   num_devices=N_CORES)

    xT = nc.dram_tensor("xT", [D, TOK], F32, kind="ExternalInput")
    wq = nc.dram_tensor("wq", [D, CW], F32, kind="ExternalInput")
    wk = nc.dram_tensor("wk", [D, CW], F32, kind="ExternalInput")
    wv = nc.dram_tensor("wv", [D, CW], F32, kind="ExternalInput")
    wo = nc.dram_tensor("wo", [D, CW], F32, kind="ExternalInput")  # row-permuted
    bq = nc.dram_tensor("bq", [CW, 1], F32, kind="ExternalInput")
    bk = nc.dram_tensor("bk", [CW, 1], F32, kind="ExternalInput")
    bv = nc.dram_tensor("bv", [CW, 1], F32, kind="ExternalInput")
    bo = nc.dram_tensor("bo", [CW, 1], F32, kind="ExternalInput")
    ident = nc.dram_tensor("ident", [128, 128], F32, kind="ExternalInput")
    outT = nc.dram_tensor("outT", [CW, TOK], F32, kind="ExternalOutput")

    with tile.TileContext(nc) as tc:
        with tc.tile_pool(name="sb", bufs=1) as sb, \
             tc.tile_pool(name="dram", bufs=1, space="DRAM") as dram:
            # ---------------- prologue: weights, biases, constants --------
            w_r = {}
            for wname, wdram, odt in (("wq", wq, F32R), ("wk", wk, F32R),
                                      ("wv", wv, F32R), ("wo", wo, BF16)):
                wr = sb.tile([128, D], odt, tag=f"{wname}_r",
                             name=f"{wname}_r")
                wsrc = wdram.ap().rearrange("(k p) m -> p k m", p=128)
                for half in range(2):
                    wst = sb.tile([128, 512], F32, tag="xstage", bufs=6,
                                  name=f"wst_{wname}{half}")
                    nc.sync.dma_start(
                        wst[:].rearrange("p (k m) -> p k m", k=4),
                        wsrc[:, half * 4:half * 4 + 4, :])
                    nc.vector.tensor_copy(
                        wr[:, half * 512:(half + 1) * 512], wst[:])
                w_r[wname] = wr
            wq_r, wk_r, wv_r, wo_b = w_r["wq"], w_r["wk"], w_r["wv"], w_r["wo"]

            idst = sb.tile([128, 512], F32, tag="xstage", bufs=6, name="idst")
            nc.sync.dma_start(idst[:, 0:128], ident[:])
            identr = sb.tile([128, 128], F32R, tag="identr", name="identr")
            nc.vector.tensor_copy(identr[:], idst[:, 0:128])

            bias_t = {}
            for bname, bdram in (("bq", bq), ("bk", bk), ("bv", bv),
                                 ("bo", bo)):
                bt_ = sb.tile([CW, 1], F32, tag=f"{bname}_t", name=f"{bname}_t")
                nc.sync.dma_start(bt_[:], bdram[:])
                bias_t[bname] = bt_

            ones_f = sb.tile([65, 64], F32, tag="ones_f", name="ones_f")
            nc.vector.memset(ones_f[:], 1.0)
            ones_r = sb.tile([65, 64], F32R, tag="ones_r", name="ones_r")
            nc.vector.tensor_copy(ones_r[:], ones_f[:])

            agin = {}
            agout = {}
            for b in range(B):
                for h in range(2):
                    for q in range(NQS):
                        agin[(b, h, q)] = dram.tile(
                            [64, QS], BF16, tag=f"agi{b}{h}{q}",
                            name=f"agi{b}{h}{q}")
                        agout[(b, h, q)] = dram.tile(
                            [64 * N_CORES, QS], BF16, tag=f"ago{b}{h}{q}",
                            addr_space="Shared", name=f"ago{b}{h}{q}")

            with tc.tile_pool(name="ps12", bufs=1, space="PSUM") as ps:
                qkv = {}
                xr_tiles = {}
                vext = {}
                at_tiles = {}

                def emit_p1_loads(b, tb):
                    if tb == 0:
                        qkv[b] = (
                            sb.tile([128, S], BF16, tag="qt_sb", bufs=2,
                                    name=f"qt{b}"),
                            sb.tile([128, S], BF16, tag="kt_sb", bufs=2,
                                    name=f"kt{b}"),
                            sb.tile([128, S], F32R, tag="vt_sb", bufs=2,
                                    name=f"vt{b}"),
                        )
                    g0 = b * S + tb * 512
                    xr = []
                    for k in range(8):
                        xs = sb.tile([128, 512], F32, tag="xstage", bufs=6,
                                     name=f"xs{b}_{tb}_{k}")
                        nc.sync.dma_start(
                            xs[:], xT[k * 128:(k + 1) * 128, g0:g0 + 512])
                        xk = sb.tile([128, 512], F32R, tag="xr", bufs=18,
                                     name=f"xr{b}_{tb}_{k}")
                        nc.vector.tensor_copy(xk[:], xs[:])
                        xr.append(xk)
                    xr_tiles[(b, tb)] = xr

                def emit_p1_group(b, tb, which):
                    qt, kt, vt = qkv[b]
                    xr = xr_tiles[(b, tb)]
                    w_, out_sb, bias = (
                        (wq_r, qt, bias_t["bq"]),
                        (wk_r, kt, bias_t["bk"]),
                        (wv_r, vt, bias_t["bv"]))[which]
                    pp = ps.tile([128, 512], F32, tag="proj", bufs=2,
                                 name=f"pp{b}_{tb}_{which}")
                    for k in range(8):
                        nc.tensor.matmul(
                            pp[:], w_[:, k * 128:(k + 1) * 128],
                            xr[k][:], start=(k == 0), stop=(k == 7))
                    nc.vector.tensor_scalar_add(
                        out_sb[:, tb * 512:(tb + 1) * 512], pp[:], bias[:])

                def emit_vext_chunk(b, tbi):
                    vt = qkv[b][2]
                    for ktc in range(4 * tbi, 4 * tbi + 4):
                        tp = ps.tile([128, 128], F32R, tag="proj", bufs=2,
                                     name=f"tp{b}_{ktc}")
                        nc.tensor.transpose(
                            tp[:], vt[:, ktc * 128:(ktc + 1) * 128],
                            identr[:])
                        for h in range(2):
                            ve = sb.tile([128, 128], BF16, tag="vext",
                                         bufs=36, name=f"ve{b}_{ktc}_{h}")
                            nc.vector.memset(ve[:, 0:64], 1.0)
                            nc.vector.tensor_copy(
                                ve[:, 64:128],
                                tp[:, h * 64:(h + 1) * 64])
                            vext[(b, ktc, h)] = ve

                def emit_p2_stripe(b, h, qs_i, jobs):
                    qt, kt, vt = qkv[b]
                    if qs_i == 0:
                        at_tiles[(b, h)] = sb.tile(
                            [128, S], BF16, tag="at_t", bufs=2,
                            name=f"at{b}_{h}")
                    at_t = at_tiles[(b, h)]
                    q0 = qs_i * QS
                    pav = ps.tile([128, QS], F32, tag="av", bufs=1,
                                  name=f"pav{b}_{h}_{qs_i}")
                    def emit_av(ktc, pt):
                        for half in range(2):
                            nc.tensor.matmul(
                                pav[:, half * 512:(half + 1) * 512],
                                vext[(b, ktc, h)][:],
                                pt[:, half * 512:(half + 1) * 512],
                                start=(ktc == 0), stop=(ktc == NKT - 1))

                    for ktc in range(NKT):
                        if ktc in jobs:
                            jobs[ktc]()
                        s_ps = ps.tile([128, QS], F32, tag="s", bufs=2,
                                       name=f"s{b}{h}{qs_i}{ktc}")
                        for half in range(2):
                            nc.tensor.matmul(
                                s_ps[:, half * 512:(half + 1) * 512],
                                kt[h * 64:(h + 1) * 64,
                                   ktc * 128:(ktc + 1) * 128],
                                qt[h * 64:(h + 1) * 64,
                                   q0 + half * 512:q0 + (half + 1) * 512],
                                start=True, stop=True,
                                tile_position=(h * 64, 0))
                        pt = sb.tile([128, QS], BF16, tag="p_sb",
                                     bufs=3, name=f"p{b}{h}{qs_i}{ktc}")
                        nc.scalar.activation(pt[:], s_ps[:], AF.Exp,
                                             scale=0.125)
                        emit_av(ktc, pt)
                    araw = sb.tile([128, QS], F32, tag="araw", bufs=3,
                                   name=f"ar{b}_{h}_{qs_i}")
                    nc.vector.tensor_copy(araw[:], pav[:])
                    rcf = sb.tile([128, QS], F32, tag="rcf", bufs=3,
                                  name=f"rcf{b}_{h}_{qs_i}")
                    nc.vector.reciprocal_approx_fast(rcf[:], araw[:])
                    bcs = sb.tile([128, QS], F32, tag="bcs", bufs=2,
                                  name=f"bcs{b}_{h}_{qs_i}")
                    nc.gpsimd.partition_broadcast(bcs[:], rcf[0:1, :])
                    nc.vector.tensor_mul(at_t[64:128, q0:q0 + QS],
                                         araw[64:128, :], bcs[64:128, :])
                    nc.sync.dma_start(agin[(b, h, qs_i)][:],
                                      at_t[64:128, q0:q0 + QS])
                    nc.gpsimd.collective_compute(
                        "AllGather", mybir.AluOpType.bypass,
                        replica_groups=[list(range(N_CORES))],
                        ins=[agin[(b, h, qs_i)][:]],
                        outs=[agout[(b, h, qs_i)][:]],
                    )

                def emit_p3_tb(tb):
                    bb = tb // 4
                    qsb = (tb % 4) // 2
                    hf = tb % 2
                    c0 = hf * 512
                    po = ps.tile([128, 512], F32, tag="proj", bufs=2,
                                 name=f"po{tb}")
                    for kc in range(8):
                        ast = sb.tile([128, 512], BF16, tag="ast", bufs=6,
                                      name=f"ast{tb}_{kc}")
                        src = agout[(bb, kc // 4, qsb)]
                        r0 = (kc % 4) * 128
                        nc.sync.dma_start(ast[:],
                                          src[r0:r0 + 128, c0:c0 + 512])
                        nc.tensor.matmul(po[:],
                                         wo_b[:, kc * 128:(kc + 1) * 128],
                                         ast[:], start=(kc == 0),
                                         stop=(kc == 7))
                    ot = sb.tile([128, 512], F32, tag="ot", bufs=3,
                                 name=f"ot{tb}")
                    nc.vector.tensor_scalar_add(ot[:], po[:], bias_t["bo"][:])
                    nc.sync.dma_start(outT[:, tb * 512:(tb + 1) * 512], ot[:])

                # batch 0 projections up front
                for tb in range(4):
                    emit_p1_loads(0, tb)
                    for w in range(3):
                        emit_p1_group(0, tb, w)
                for tbi in range(4):
                    emit_vext_chunk(0, tbi)

                for b in range(B):
                    for h in range(2):
                        for qs_i in range(NQS):
                            i = h * NQS + qs_i
                            jobs = {}
                            # P3 token-blocks woven into later batches' stripes
                            p3_sched = {(1, 1): [0], (1, 2): [1], (1, 3): [2],
                                        (2, 0): [3], (2, 1): [4], (2, 2): [5],
                                        (2, 3): [6], (3, 0): [7],
                                        (3, 1): [8, 9], (3, 2): [10, 11],
                                        (3, 3): [12, 13]}
                            for j, tb3 in enumerate(p3_sched.get((b, i), [])):
                                jobs[5 + 8 * j] = (lambda tb=tb3:
                                                   emit_p3_tb(tb))
                            if b + 1 < B:
                                jobs[0] = (lambda bb=b + 1, tb=i:
                                           emit_p1_loads(bb, tb))
                                if i >= 1:
                                    jobs[3] = (lambda bb=b + 1, tb=i - 1:
                                               emit_p1_group(bb, tb, 0))
                                    jobs[7] = (lambda bb=b + 1, tb=i - 1:
                                               emit_p1_group(bb, tb, 1))
                                    jobs[11] = (lambda bb=b + 1, tb=i - 1:
                                                emit_p1_group(bb, tb, 2))
                                if i >= 2:
                                    jobs[14] = (lambda bb=b + 1, tbi=i - 2:
                                                emit_vext_chunk(bb, tbi))
                            emit_p2_stripe(b, h, qs_i, jobs)
                    if b + 1 < B:
                        # tail of next batch's projections
                        for w in range(3):
                            emit_p1_group(b + 1, 3, w)
                        emit_vext_chunk(b + 1, 2)
                        emit_vext_chunk(b + 1, 3)

                # ------------- P3 tail: last token blocks -----------------
                for tb in range(14, NTB):
                    emit_p3_tb(tb)

    nc.compile()
    return nc


def _get_nc():
    if "nc" not in _CACHE:
        _CACHE["nc"] = _build()
    return _CACHE["nc"]


def _make_in_maps(x, Wq, bq, Wk, bk, Wv, bv, Wo, bo):
    x = np.asarray(x, dtype=np.float32)
    Wq, Wk, Wv, Wo = (np.asarray(w, dtype=np.float32) for w in (Wq, Wk, Wv, Wo))
    bq, bk, bv, bo = (np.asarray(v, dtype=np.float32) for v in (bq, bk, bv, bo))

    xT = np.ascontiguousarray(x.reshape(TOK, D).T)
    # Wo rows permuted: gathered A'^T row r*64+t of head-block h corresponds
    # to head (2r+h), dim t -> original Wo row r*128 + h*64 + t.
    wo4 = Wo.reshape(N_CORES, 2, HD, D)
    wo_perm = np.concatenate([wo4[:, 0], wo4[:, 1]], axis=0).reshape(D, D)

    in_maps = []
    for c in range(N_CORES):
        cs = slice(c * CW, (c + 1) * CW)
        in_maps.append({
            "xT": xT,
            "wq": np.ascontiguousarray(Wq[:, cs]),
            "wk": np.ascontiguousarray(Wk[:, cs]),
            "wv": np.ascontiguousarray(Wv[:, cs]),
            "wo": np.ascontiguousarray(wo_perm[:, cs]),
            "bq": np.ascontiguousarray(bq[cs].reshape(CW, 1)),
            "bk": np.ascontiguousarray(bk[cs].reshape(CW, 1)),
            "bv": np.ascontiguousarray(bv[cs].reshape(CW, 1)),
            "bo": np.ascontiguousarray(bo[cs].reshape(CW, 1)),
            "ident": np.eye(128, dtype=np.float32),
        })
    return in_maps


def kernel(x, Wq, bq, Wk, bk, Wv, bv, Wo, bo):
    from concourse import bass_utils

    in_maps = _make_in_maps(x, Wq, bq, Wk, bk, Wv, bv, Wo, bo)
    nc = _get_nc()
    res = bass_utils.run_bass_kernel_spmd(nc, in_maps,
                                          core_ids=list(range(N_CORES)))
    _CACHE["last_results"] = res

    out = np.empty((TOK, D), dtype=np.float32)
    for c in range(N_CORES):
        out[:, c * CW:(c + 1) * CW] = res.results[c]["outT"].T
    return out.reshape(B, S, D)



# revision 16
# speedup vs baseline: 1.0993x; 1.0993x over previous
"""Multi-head self-attention (B=4, S=2048, D=1024, H=16, Hd=64) on 8 TRN2 cores.

Sharding: tensor-parallel over heads for QKV+attention (core c owns heads
2c, 2c+1), token-parallel for the output projection (core c owns tokens
[b*2048 + hf*1024 + c*128, +128) for each half hf), bridged by two small
AllToAlls per batch (256 KB/rank, fired at mid-batch and batch-end) so
communication always completes a full batch before its consumers run.

All-bf16 datapath: x and weights are converted to bf16 on the host and
DMA'd directly into SBUF (no on-device casts). Per (batch, 512-query
stripe):
  - S^T pair: both heads' score matmuls issued back-to-back with
    tile_position (0,0)/(64,0) -> concurrent on the PE array (each uses
    only 64 contraction rows), into one [128,1024] PSUM tile.
  - one exp on ACT over both heads' scores ([128,1024], scale=1/8 folded).
  - AV per head with ve = [ones(64) | V(64)] so the softmax denominator
    lands in PSUM rows 0:64 and the AV rows in 64:128 for BOTH heads:
    every DVE op and the single partition_broadcast stay base-aligned.
  - normalize on DVE into per-head A^T tiles (rows 64:128 used).
Out-proj: lhsT = full (row-permuted) Wo, rhs = AllToAll-gathered A'^T
chunks, woven into the next batch's attention stripes; batch 3's halves
are consumed separately so the tail only waits for the last 256 KB A2A.
"""
import numpy as np

B, S, D, H, HD = 4, 2048, 1024, 16, 64
N_CORES = 8
TOK = B * S            # 8192
HPC = H // N_CORES     # 2 heads per core
CW = HPC * HD          # 128 cols per core
QS = 512               # query stripe
NKT = S // 128         # 16 kt chunks per batch
NQS = S // QS          # 4 q stripes per batch
TPC = S // N_CORES     # 256 tokens per (core, batch)
HP = TPC // 2          # 128 tokens per (core, batch, half)

_CACHE = {}


def _build():
    import concourse.bacc as bacc
    import concourse.mybir as mybir
    import concourse.tile as tile

    F32 = mybir.dt.float32
    F32R = mybir.dt.float32r
    BF16 = mybir.dt.bfloat16
    AF = mybir.ActivationFunctionType

    nc = bacc.Bacc(trn_type="TRN2", target_bir_lowering=False, debug=False,
                   num_devices=N_CORES)

    xT = nc.dram_tensor("xT", [D, TOK], BF16, kind="ExternalInput")
    wq = nc.dram_tensor("wq", [128, D], BF16, kind="ExternalInput")
    wk = nc.dram_tensor("wk", [128, D], BF16, kind="ExternalInput")
    wv = nc.dram_tensor("wv", [128, D], BF16, kind="ExternalInput")
    wo = nc.dram_tensor("wo", [128, 8 * D], BF16, kind="ExternalInput")
    bq = nc.dram_tensor("bq", [CW, 1], F32, kind="ExternalInput")
    bk = nc.dram_tensor("bk", [CW, 1], F32, kind="ExternalInput")
    bv = nc.dram_tensor("bv", [CW, 1], F32, kind="ExternalInput")
    bo = nc.dram_tensor("bo", [128, 8], F32, kind="ExternalInput")
    ident = nc.dram_tensor("ident", [128, 128], F32R, kind="ExternalInput")
    outT = nc.dram_tensor("outT", [D, B * TPC], F32, kind="ExternalOutput")

    with tile.TileContext(nc) as tc:
        with tc.tile_pool(name="sb", bufs=1) as sb, \
             tc.tile_pool(name="dram", bufs=1, space="DRAM") as dram:
            # ---------------- prologue: small weights, biases, constants --
            wq_s = sb.tile([128, D], BF16, tag="wq_s", name="wq_s")
            nc.sync.dma_start(wq_s[:], wq[:])
            wk_s = sb.tile([128, D], BF16, tag="wk_s", name="wk_s")
            nc.sync.dma_start(wk_s[:], wk[:])
            wv_s = sb.tile([128, D], BF16, tag="wv_s", name="wv_s")
            nc.sync.dma_start(wv_s[:], wv[:])
            identr = sb.tile([128, 128], F32R, tag="identr", name="identr")
            nc.sync.dma_start(identr[:], ident[:])

            bias_t = {}
            for bname, bdram, bshape in (("bq", bq, [CW, 1]),
                                         ("bk", bk, [CW, 1]),
                                         ("bv", bv, [CW, 1]),
                                         ("bo", bo, [128, 8])):
                bt_ = sb.tile(bshape, F32, tag=f"{bname}_t", name=f"{bname}_t")
                nc.sync.dma_start(bt_[:], bdram[:])
                bias_t[bname] = bt_

            a2a_in = {}
            a2a_out = {}
            for b in range(B - 1):
                for hf in range(2):
                    a2a_in[(b, hf)] = dram.tile(
                        [N_CORES * 128, HP], BF16,
                        tag=f"a2ai{b}{hf}", name=f"a2ai{b}{hf}")
                    a2a_out[(b, hf)] = dram.tile(
                        [N_CORES * 128, HP], BF16,
                        tag=f"a2ao{b}{hf}", name=f"a2ao{b}{hf}")
            QP = HP // 2  # 64-token quarters for batch 3
            for qq in range(4):
                a2a_in[(3, qq)] = dram.tile(
                    [N_CORES * 128, QP], BF16,
                    tag=f"a2ai3{qq}", name=f"a2ai3{qq}")
                a2a_out[(3, qq)] = dram.tile(
                    [N_CORES * 128, QP], BF16,
                    tag=f"a2ao3{qq}", name=f"a2ao3{qq}")

            with tc.tile_pool(name="ps12", bufs=1, space="PSUM") as ps:
                qkv = {}
                xr_tiles = {}
                vext = {}
                at_tiles = {}
                ast_tiles = {}

                def emit_p1_loads(b, tb):
                    if tb == 0:
                        qkv[b] = (
                            sb.tile([128, S], BF16, tag="qt_sb", bufs=2,
                                    name=f"qt{b}"),
                            sb.tile([128, S], BF16, tag="kt_sb", bufs=2,
                                    name=f"kt{b}"),
                            sb.tile([128, S], F32R, tag="vt_sb", bufs=2,
                                    name=f"vt{b}"),
                        )
                        at_tiles[b] = (
                            sb.tile([128, S], BF16, tag="at0",
                                    bufs=2, name=f"at0_{b}"),
                            sb.tile([128, S], BF16, tag="at1",
                                    bufs=2, name=f"at1_{b}"),
                        )
                    g0 = b * S + tb * 512
                    xr = []
                    for k in range(8):
                        xk = sb.tile([128, 512], BF16, tag="xr", bufs=34,
                                     name=f"xr{b}_{tb}_{k}")
                        nc.sync.dma_start(
                            xk[:], xT[k * 128:(k + 1) * 128, g0:g0 + 512])
                        xr.append(xk)
                    xr_tiles[(b, tb)] = xr

                def emit_p1_group(b, tb, which):
                    qt, kt, vt = qkv[b]
                    xr = xr_tiles[(b, tb)]
                    w_, out_sb, bias = (
                        (wq_s, qt, bias_t["bq"]),
                        (wk_s, kt, bias_t["bk"]),
                        (wv_s, vt, bias_t["bv"]))[which]
                    pp = ps.tile([128, 512], F32, tag="proj", bufs=2,
                                 name=f"pp{b}_{tb}_{which}")
                    for k in range(8):
                        nc.tensor.matmul(
                            pp[:], w_[:, k * 128:(k + 1) * 128],
                            xr[k][:], start=(k == 0), stop=(k == 7))
                    nc.vector.tensor_scalar_add(
                        out_sb[:, tb * 512:(tb + 1) * 512], pp[:], bias[:])

                def emit_vext_chunk(b, tbi):
                    vt = qkv[b][2]
                    for ktc in range(4 * tbi, 4 * tbi + 4):
                        tp = ps.tile([128, 128], F32R, tag="proj", bufs=2,
                                     name=f"tp{b}_{ktc}")
                        nc.tensor.transpose(
                            tp[:], vt[:, ktc * 128:(ktc + 1) * 128],
                            identr[:])
                        # both heads: [ones | V] -> denom rows 0:64,
                        # AV rows 64:128 (everything stays base-aligned)
                        ve = sb.tile([128, 128], BF16, tag="vext",
                                     bufs=36, name=f"ve{b}_{ktc}")
                        nc.gpsimd.memset(ve[:, 0:64], 1.0)
                        nc.vector.tensor_copy(ve[:, 64:128], tp[:, 0:64])
                        ve2 = sb.tile([128, 128], BF16, tag="vext",
                                      bufs=36, name=f"v2{b}_{ktc}")
                        nc.gpsimd.memset(ve2[:, 0:64], 1.0)
                        nc.vector.tensor_copy(ve2[:, 64:128], tp[:, 64:128])
                        vext[(b, ktc, 0)] = ve
                        vext[(b, ktc, 1)] = ve2

                def emit_p2_stripe(b, qs_i, jobs):
                    qt, kt, vt = qkv[b]
                    at0, at1 = at_tiles[b]
                    q0 = qs_i * QS
                    pav = ps.tile([128, 1024], F32, tag="av", bufs=1,
                                  name=f"pav{b}_{qs_i}")
                    for ktc in range(NKT):
                        for job in jobs.get(ktc, ()):
                            job()
                        s_ps = ps.tile([128, 1024], F32, tag="s", bufs=2,
                                       name=f"s{b}_{qs_i}_{ktc}")
                        nc.tensor.matmul(
                            s_ps[:, 0:512],
                            kt[0:64, ktc * 128:(ktc + 1) * 128],
                            qt[0:64, q0:q0 + 512],
                            start=True, stop=True, tile_position=(0, 0))
                        nc.tensor.matmul(
                            s_ps[:, 512:1024],
                            kt[64:128, ktc * 128:(ktc + 1) * 128],
                            qt[64:128, q0:q0 + 512],
                            start=True, stop=True, tile_position=(64, 0))
                        pt = sb.tile([128, 1024], BF16, tag="p_sb",
                                     bufs=3, name=f"p{b}_{qs_i}_{ktc}")
                        nc.scalar.activation(pt[:], s_ps[:], AF.Exp,
                                             scale=0.125)
                        nc.tensor.matmul(
                            pav[:, 0:512], vext[(b, ktc, 0)][:],
                            pt[:, 0:512],
                            start=(ktc == 0), stop=(ktc == NKT - 1))
                        nc.tensor.matmul(
                            pav[:, 512:1024], vext[(b, ktc, 1)][:],
                            pt[:, 512:1024],
                            start=(ktc == 0), stop=(ktc == NKT - 1))
                    # pav rows 0:64 = denominators, rows 64:128 = AV
                    # (cols 0:512 = h0, cols 512:1024 = h1)
                    araw = sb.tile([128, 1024], F32, tag="araw", bufs=2,
                                   name=f"ar{b}_{qs_i}")
                    nc.vector.tensor_copy(araw[:], pav[:])
                    rcf = sb.tile([128, 1024], F32, tag="rcf", bufs=2,
                                  name=f"rcf{b}_{qs_i}")
                    nc.vector.reciprocal_approx_fast(rcf[:], araw[:])
                    bcs = sb.tile([128, 1024], F32, tag="bcs", bufs=2,
                                  name=f"bcs{b}_{qs_i}")
                    nc.gpsimd.partition_broadcast(bcs[:], rcf[0:1, :])
                    nc.vector.tensor_mul(at0[64:128, q0:q0 + QS],
                                         araw[64:128, 0:512],
                                         bcs[64:128, 0:512])
                    nc.vector.tensor_mul(at1[64:128, q0:q0 + QS],
                                         araw[64:128, 512:1024],
                                         bcs[64:128, 512:1024])
                    # rows r<64 of a shard = h1 (head 2c+1), r>=64 = h0
                    if b < B - 1:
                        hf = qs_i // 2
                        for jj in range(4):
                            j = (qs_i % 2) * 4 + jj
                            tok = q0 + jj * HP
                            nc.gpsimd.dma_start(
                                a2a_in[(b, hf)][j * 128:j * 128 + 64, :],
                                at1[64:128, tok:tok + HP])
                            nc.gpsimd.dma_start(
                                a2a_in[(b, hf)][j * 128 + 64:(j + 1) * 128, :],
                                at0[64:128, tok:tok + HP])
                        if qs_i % 2 == 1:
                            nc.gpsimd.collective_compute(
                                "AllToAll", mybir.AluOpType.bypass,
                                replica_groups=[list(range(N_CORES))],
                                ins=[a2a_in[(b, hf)][:]],
                                outs=[a2a_out[(b, hf)][:]],
                            )
                    else:
                        # batch 3: one quarter-A2A per stripe
                        for j in range(8):
                            tok = q0 + j * QP
                            nc.gpsimd.dma_start(
                                a2a_in[(3, qs_i)][j * 128:j * 128 + 64, :],
                                at1[64:128, tok:tok + QP])
                            nc.gpsimd.dma_start(
                                a2a_in[(3, qs_i)][j * 128 + 64:(j + 1) * 128, :],
                                at0[64:128, tok:tok + QP])
                        nc.gpsimd.collective_compute(
                            "AllToAll", mybir.AluOpType.bypass,
                            replica_groups=[list(range(N_CORES))],
                            ins=[a2a_in[(3, qs_i)][:]],
                            outs=[a2a_out[(3, qs_i)][:]],
                        )

                def emit_ast_loads(b, parts):
                    ast = ast_tiles.get(b)
                    if ast is None:
                        ast = [sb.tile([128, TPC], BF16, tag="ast", bufs=18,
                                       name=f"ast{b}_{k}") for k in range(8)]
                        ast_tiles[b] = ast
                    w = HP if b < B - 1 else QP
                    for hf in parts:
                        for k in range(8):
                            nc.sync.dma_start(
                                ast[k][:, hf * w:(hf + 1) * w],
                                a2a_out[(b, hf)][k * 128:(k + 1) * 128, :])

                def emit_p3_ogroup(b, o, hf=None):
                    ast = ast_tiles[b]
                    w = HP if b < B - 1 else QP
                    c0, nc_ = (0, TPC) if hf is None else (hf * w, w)
                    po = ps.tile([128, TPC], F32, tag="proj", bufs=2,
                                 name=f"po{b}_{o}_{hf}")
                    for k in range(8):
                        nc.tensor.matmul(
                            po[:, 0:nc_],
                            wo_s[:, k * D + o * 128:k * D + (o + 1) * 128],
                            ast[k][:, c0:c0 + nc_],
                            start=(k == 0), stop=(k == 7))
                    ot = sb.tile([128, TPC], F32, tag="ot", bufs=3,
                                 name=f"ot{b}_{o}_{hf}")
                    nc.vector.tensor_scalar_add(ot[:, 0:nc_], po[:, 0:nc_],
                                                bias_t["bo"][:, o:o + 1])
                    nc.gpsimd.dma_start(
                        outT[o * 128:(o + 1) * 128,
                             b * TPC + c0:b * TPC + c0 + nc_],
                        ot[:, 0:nc_])

                # ---------------- batch 0 fast-start ----------------------
                emit_p1_loads(0, 0)
                emit_p1_loads(0, 1)
                for w in (1, 2, 0):
                    emit_p1_group(0, 0, w)
                emit_vext_chunk(0, 0)
                # big Wo DMA deferred so batch-0 x loads win the queues
                wo_s = sb.tile([128, 8 * D], BF16, tag="wo_s", name="wo_s")
                for q in range(4):
                    nc.sync.dma_start(wo_s[:, q * 2048:(q + 1) * 2048],
                                      wo[:, q * 2048:(q + 1) * 2048])

                def sched(b):
                    """jobs[qs][ktc] for stripes of batch b."""
                    jobs = {qs: {} for qs in range(NQS)}

                    def add(qs, ktc, fn):
                        jobs[qs].setdefault(ktc, []).append(fn)

                    nb = b + 1
                    if b == 0:
                        # finish batch 0's own projections inside stripes 0/1
                        add(0, 0, lambda: emit_p1_group(0, 1, 1))
                        add(0, 2, lambda: emit_p1_group(0, 1, 2))
                        add(0, 3, lambda: emit_vext_chunk(0, 1))
                        add(0, 4, lambda: emit_p1_loads(0, 2))
                        add(0, 6, lambda: emit_p1_group(0, 2, 1))
                        add(0, 7, lambda: emit_p1_group(0, 2, 2))
                        add(0, 8, lambda: emit_vext_chunk(0, 2))
                        add(0, 9, lambda: emit_p1_loads(0, 3))
                        add(0, 10, lambda: emit_p1_group(0, 3, 1))
                        add(0, 11, lambda: emit_p1_group(0, 3, 2))
                        add(0, 12, lambda: emit_vext_chunk(0, 3))
                        add(0, 14, lambda: emit_p1_group(0, 1, 0))
                        add(1, 1, lambda: emit_p1_group(0, 2, 0))
                        add(1, 3, lambda: emit_p1_group(0, 3, 0))
                        add(1, 6, lambda: emit_p1_loads(1, 0))
                        add(1, 9, lambda: emit_p1_loads(1, 1))
                        add(1, 12, lambda: emit_p1_group(1, 0, 0))
                        add(1, 15, lambda: emit_p1_group(1, 0, 1))
                        add(2, 1, lambda: emit_p1_group(1, 0, 2))
                        add(2, 4, lambda: emit_p1_group(1, 1, 0))
                        add(2, 7, lambda: emit_p1_group(1, 1, 1))
                        add(2, 10, lambda: emit_p1_group(1, 1, 2))
                        add(2, 13, lambda: emit_p1_loads(1, 2))
                        add(2, 15, lambda: emit_vext_chunk(1, 0))
                        add(3, 1, lambda: emit_p1_group(1, 2, 0))
                        add(3, 3, lambda: emit_p1_group(1, 2, 1))
                        add(3, 5, lambda: emit_p1_group(1, 2, 2))
                        add(3, 7, lambda: emit_p1_loads(1, 3))
                        add(3, 9, lambda: emit_p1_group(1, 3, 0))
                        add(3, 11, lambda: emit_p1_group(1, 3, 1))
                        add(3, 13, lambda: emit_p1_group(1, 3, 2))
                        add(3, 15, lambda: emit_vext_chunk(1, 1))
                        return jobs

                    # b >= 1: weave next batch's projections + prev batch's
                    # out-proj (both halves landed a full batch ago).
                    add(0, 0, lambda pb=b - 1: emit_ast_loads(pb, (0, 1)))
                    for i in range(4):
                        add(1, 1 + 4 * i,
                            lambda pb=b - 1, o=i: emit_p3_ogroup(pb, o))
                        add(2, 1 + 4 * i,
                            lambda pb=b - 1, o=4 + i: emit_p3_ogroup(pb, o))
                    if nb < B:
                        add(0, 1, lambda: emit_p1_loads(nb, 0))
                        add(0, 4, lambda: emit_p1_loads(nb, 1))
                        add(0, 7, lambda: emit_p1_group(nb, 0, 0))
                        add(0, 10, lambda: emit_p1_group(nb, 0, 1))
                        add(0, 13, lambda: emit_p1_group(nb, 0, 2))
                        add(1, 3, lambda: emit_p1_group(nb, 1, 0))
                        add(1, 7, lambda: emit_p1_group(nb, 1, 1))
                        add(1, 11, lambda: emit_p1_group(nb, 1, 2))
                        add(1, 14, lambda: emit_p1_loads(nb, 2))
                        add(2, 3, lambda: emit_p1_group(nb, 2, 0))
                        add(2, 7, lambda: emit_p1_group(nb, 2, 1))
                        add(2, 11, lambda: emit_p1_group(nb, 2, 2))
                        add(2, 14, lambda: emit_p1_loads(nb, 3))
                        add(2, 15, lambda: emit_vext_chunk(nb, 0))
                        add(3, 1, lambda: emit_p1_group(nb, 3, 0))
                        add(3, 3, lambda: emit_p1_group(nb, 3, 1))
                        add(3, 5, lambda: emit_p1_group(nb, 3, 2))
                        add(3, 8, lambda: emit_vext_chunk(nb, 1))
                        add(3, 11, lambda: emit_vext_chunk(nb, 2))
                        add(3, 14, lambda: emit_vext_chunk(nb, 3))
                    else:
                        # batch 3: quarter q's out-proj woven into stripe q+1
                        for q in range(3):
                            add(q + 1, 2, lambda qq=q: emit_ast_loads(3, (qq,)))
                            for i in range(8):
                                add(q + 1, 4 + int(i * 1.45),
                                    lambda qq=q, o=i: emit_p3_ogroup(
                                        3, o, hf=qq))
                    return jobs

                for b in range(B):
                    jobs = sched(b)
                    for qs_i in range(NQS):
                        emit_p2_stripe(b, qs_i, jobs[qs_i])
                    if b == 0:
                        # b0 fast-start didn't have room in qs3 for these
                        emit_vext_chunk(1, 2)
                        emit_vext_chunk(1, 3)

                # ------------- tail: batch 3 last-quarter out-proj --------
                emit_ast_loads(3, (3,))
                for o in range(8):
                    emit_p3_ogroup(3, o, hf=3)

    nc.compile()
    return nc


def _get_nc():
    if "nc" not in _CACHE:
        _CACHE["nc"] = _build()
    return _CACHE["nc"]


def _make_in_maps(x, Wq, bq, Wk, bk, Wv, bv, Wo, bo):
    import ml_dtypes
    bf16 = ml_dtypes.bfloat16

    x = np.asarray(x, dtype=np.float32)
    Wq, Wk, Wv, Wo = (np.asarray(w, dtype=np.float32) for w in (Wq, Wk, Wv, Wo))
    bq, bk, bv, bo = (np.asarray(v, dtype=np.float32) for v in (bq, bk, bv, bo))

    xT = np.ascontiguousarray(x.reshape(TOK, D).T.astype(bf16))

    def warr(W, cs):
        # [128, 8*128]: chunk k cols <- W[k*128:(k+1)*128, cs]
        return np.ascontiguousarray(
            W[:, cs].reshape(8, 128, CW).transpose(1, 0, 2).reshape(128, D)
            .astype(bf16))

    # A2A-gathered A'^T row 128*i + r: r < 64 -> head 2i+1, r >= 64 ->
    # head 2i, dim r % 64.
    perm = np.empty(D, dtype=np.int64)
    for i in range(8):
        for r in range(128):
            h = 2 * i + (1 if r < 64 else 0)
            perm[128 * i + r] = h * 64 + (r % 64)
    wo_p = Wo[perm]  # [1024, 1024]
    wo_host = np.ascontiguousarray(
        wo_p.reshape(8, 128, D).transpose(1, 0, 2).reshape(128, 8 * D)
        .astype(bf16))

    bo_host = np.ascontiguousarray(bo.reshape(8, 128).T)

    in_maps = []
    for c in range(N_CORES):
        cs = slice(c * CW, (c + 1) * CW)
        in_maps.append({
            "xT": xT,
            "wq": warr(Wq, cs),
            "wk": warr(Wk, cs),
            "wv": warr(Wv, cs),
            "wo": wo_host,
            "bq": np.ascontiguousarray(bq[cs].reshape(CW, 1)),
            "bk": np.ascontiguousarray(bk[cs].reshape(CW, 1)),
            "bv": np.ascontiguousarray(bv[cs].reshape(CW, 1)),
            "bo": bo_host,
            "ident": np.eye(128, dtype=np.float32),
        })
    return in_maps


def kernel(x, Wq, bq, Wk, bk, Wv, bv, Wo, bo):
    from concourse import bass_utils

    in_maps = _make_in_maps(x, Wq, bq, Wk, bk, Wv, bv, Wo, bo)
    nc = _get_nc()
    res = bass_utils.run_bass_kernel_spmd(nc, in_maps,
                                          core_ids=list(range(N_CORES)))
    _CACHE["last_results"] = res

    out = np.empty((B, S, D), dtype=np.float32)
    QP = HP // 2
    for c in range(N_CORES):
        r = res.results[c]["outT"]  # [1024 odim, 4*256 token slots]
        for b in range(B - 1):
            for hf in range(2):
                t0 = b * S + hf * (S // 2) + c * HP
                out.reshape(TOK, D)[t0:t0 + HP, :] = \
                    r[:, b * TPC + hf * HP:b * TPC + (hf + 1) * HP].T
        for qq in range(4):
            t0 = 3 * S + qq * QS + c * QP
            out.reshape(TOK, D)[t0:t0 + QP, :] = \
                r[:, 3 * TPC + qq * QP:3 * TPC + (qq + 1) * QP].T
    return out


# revision 17
# speedup vs baseline: 1.1554x; 1.0510x over previous
"""Multi-head self-attention (B=4, S=2048, D=1024, H=16, Hd=64) on 8 TRN2 cores.

Sharding: tensor-parallel over heads for QKV+attention (core c owns heads
2c, 2c+1), token-parallel for the output projection (core c owns tokens
[b*2048 + hf*1024 + c*128, +128) for each half hf), bridged by two small
AllToAlls per batch (256 KB/rank, fired at mid-batch and batch-end) so
communication always completes a full batch before its consumers run.

All-bf16 datapath: x and weights are converted to bf16 on the host and
DMA'd directly into SBUF (no on-device casts). Per (batch, 512-query
stripe):
  - S^T pair: both heads' score matmuls issued back-to-back with
    tile_position (0,0)/(64,0) -> concurrent on the PE array (each uses
    only 64 contraction rows), into one [128,1024] PSUM tile.
  - one exp on ACT over both heads' scores ([128,1024], scale=1/8 folded).
  - AV per head with ve = [ones(64) | V(64)] so the softmax denominator
    lands in PSUM rows 0:64 and the AV rows in 64:128 for BOTH heads:
    every DVE op and the single partition_broadcast stay base-aligned.
  - normalize on DVE into per-head A^T tiles (rows 64:128 used).
Out-proj: lhsT = full (row-permuted) Wo, rhs = AllToAll-gathered A'^T
chunks, woven into the next batch's attention stripes; batch 3's halves
are consumed separately so the tail only waits for the last 256 KB A2A.
"""
import numpy as np

B, S, D, H, HD = 4, 2048, 1024, 16, 64
N_CORES = 8
TOK = B * S            # 8192
HPC = H // N_CORES     # 2 heads per core
CW = HPC * HD          # 128 cols per core
QS = 512               # query stripe
NKT = S // 128         # 16 kt chunks per batch
NQS = S // QS          # 4 q stripes per batch
TPC = S // N_CORES     # 256 tokens per (core, batch)
HP = TPC // 2          # 128 tokens per (core, batch, half)

_CACHE = {}


def _build():
    import concourse.bacc as bacc
    import concourse.mybir as mybir
    import concourse.tile as tile

    F32 = mybir.dt.float32
    F32R = mybir.dt.float32r
    BF16 = mybir.dt.bfloat16
    AF = mybir.ActivationFunctionType

    nc = bacc.Bacc(trn_type="TRN2", target_bir_lowering=False, debug=False,
                   num_devices=N_CORES)

    xT = nc.dram_tensor("xT", [D, TOK], BF16, kind="ExternalInput")
    wq = nc.dram_tensor("wq", [128, D], BF16, kind="ExternalInput")
    wk = nc.dram_tensor("wk", [128, D], BF16, kind="ExternalInput")
    wv = nc.dram_tensor("wv", [128, D], BF16, kind="ExternalInput")
    wo = nc.dram_tensor("wo", [128, 8 * D], BF16, kind="ExternalInput")
    bq = nc.dram_tensor("bq", [CW, 1], F32, kind="ExternalInput")
    bk = nc.dram_tensor("bk", [CW, 1], F32, kind="ExternalInput")
    bv = nc.dram_tensor("bv", [CW, 1], F32, kind="ExternalInput")
    bo = nc.dram_tensor("bo", [128, 8], F32, kind="ExternalInput")
    ident = nc.dram_tensor("ident", [128, 128], F32R, kind="ExternalInput")
    outT = nc.dram_tensor("outT", [D, B * TPC], F32, kind="ExternalOutput")

    with tile.TileContext(nc) as tc:
        with tc.tile_pool(name="sb", bufs=1) as sb, \
             tc.tile_pool(name="dram", bufs=1, space="DRAM") as dram:
            # ---------------- prologue: small weights, biases, constants --
            wq_s = sb.tile([128, D], BF16, tag="wq_s", name="wq_s")
            nc.sync.dma_start(wq_s[:], wq[:])
            wk_s = sb.tile([128, D], BF16, tag="wk_s", name="wk_s")
            nc.sync.dma_start(wk_s[:], wk[:])
            wv_s = sb.tile([128, D], BF16, tag="wv_s", name="wv_s")
            nc.sync.dma_start(wv_s[:], wv[:])
            identr = sb.tile([128, 128], F32R, tag="identr", name="identr")
            nc.sync.dma_start(identr[:], ident[:])

            bias_t = {}
            for bname, bdram, bshape in (("bq", bq, [CW, 1]),
                                         ("bk", bk, [CW, 1]),
                                         ("bv", bv, [CW, 1]),
                                         ("bo", bo, [128, 8])):
                bt_ = sb.tile(bshape, F32, tag=f"{bname}_t", name=f"{bname}_t")
                nc.sync.dma_start(bt_[:], bdram[:])
                bias_t[bname] = bt_

            a2a_in = {}
            a2a_out = {}
            for b in range(B - 1):
                for hf in range(2):
                    a2a_in[(b, hf)] = dram.tile(
                        [N_CORES * 128, HP], BF16,
                        tag=f"a2ai{b}{hf}", name=f"a2ai{b}{hf}")
                    a2a_out[(b, hf)] = dram.tile(
                        [N_CORES * 128, HP], BF16,
                        tag=f"a2ao{b}{hf}", name=f"a2ao{b}{hf}")
            QP = HP // 2  # 64-token quarters for batch 3
            for qq in range(4):
                a2a_in[(3, qq)] = dram.tile(
                    [N_CORES * 128, QP], BF16,
                    tag=f"a2ai3{qq}", name=f"a2ai3{qq}")
                a2a_out[(3, qq)] = dram.tile(
                    [N_CORES * 128, QP], BF16,
                    tag=f"a2ao3{qq}", name=f"a2ao3{qq}")

            with tc.tile_pool(name="ps12", bufs=1, space="PSUM") as ps:
                qkv = {}
                xr_tiles = {}
                vext = {}
                at_tiles = {}
                ast_tiles = {}

                def emit_p1_loads(b, tb):
                    if tb == 0:
                        qkv[b] = (
                            sb.tile([128, S], BF16, tag="qt_sb", bufs=2,
                                    name=f"qt{b}"),
                            sb.tile([128, S], BF16, tag="kt_sb", bufs=2,
                                    name=f"kt{b}"),
                            sb.tile([128, S], F32R, tag="vt_sb", bufs=2,
                                    name=f"vt{b}"),
                        )
                        at_tiles[b] = (
                            sb.tile([128, S], BF16, tag="at0",
                                    bufs=2, name=f"at0_{b}"),
                            sb.tile([128, S], BF16, tag="at1",
                                    bufs=2, name=f"at1_{b}"),
                        )
                    g0 = b * S + tb * 512
                    xr = []
                    for k in range(8):
                        xk = sb.tile([128, 512], BF16, tag="xr", bufs=34,
                                     name=f"xr{b}_{tb}_{k}")
                        nc.sync.dma_start(
                            xk[:], xT[k * 128:(k + 1) * 128, g0:g0 + 512])
                        xr.append(xk)
                    xr_tiles[(b, tb)] = xr

                def emit_p1_group(b, tb, which):
                    qt, kt, vt = qkv[b]
                    xr = xr_tiles[(b, tb)]
                    w_, out_sb, bias = (
                        (wq_s, qt, bias_t["bq"]),
                        (wk_s, kt, bias_t["bk"]),
                        (wv_s, vt, bias_t["bv"]))[which]
                    pp = ps.tile([128, 512], F32, tag="proj", bufs=2,
                                 name=f"pp{b}_{tb}_{which}")
                    for k in range(8):
                        nc.tensor.matmul(
                            pp[:], w_[:, k * 128:(k + 1) * 128],
                            xr[k][:], start=(k == 0), stop=(k == 7))
                    nc.vector.tensor_scalar_add(
                        out_sb[:, tb * 512:(tb + 1) * 512], pp[:], bias[:])

                def emit_vext_chunk(b, tbi):
                    vt = qkv[b][2]
                    for ktc in range(4 * tbi, 4 * tbi + 4):
                        tp = ps.tile([128, 128], F32R, tag="proj", bufs=2,
                                     name=f"tp{b}_{ktc}")
                        nc.tensor.transpose(
                            tp[:], vt[:, ktc * 128:(ktc + 1) * 128],
                            identr[:])
                        # both heads: [ones | V] -> denom rows 0:64,
                        # AV rows 64:128 (everything stays base-aligned)
                        ve = sb.tile([128, 128], BF16, tag="vext",
                                     bufs=36, name=f"ve{b}_{ktc}")
                        nc.gpsimd.memset(ve[:, 0:64], 1.0)
                        nc.vector.tensor_copy(ve[:, 64:128], tp[:, 0:64])
                        ve2 = sb.tile([128, 128], BF16, tag="vext",
                                      bufs=36, name=f"v2{b}_{ktc}")
                        nc.gpsimd.memset(ve2[:, 0:64], 1.0)
                        nc.vector.tensor_copy(ve2[:, 64:128], tp[:, 64:128])
                        vext[(b, ktc, 0)] = ve
                        vext[(b, ktc, 1)] = ve2

                def emit_p2_stripe(b, qs_i, jobs):
                    qt, kt, vt = qkv[b]
                    at0, at1 = at_tiles[b]
                    q0 = qs_i * QS
                    pav = ps.tile([128, 1024], F32, tag="av", bufs=1,
                                  name=f"pav{b}_{qs_i}")
                    for ktc in range(NKT):
                        for job in jobs.get(ktc, ()):
                            job()
                        s_ps = ps.tile([128, 1024], F32, tag="s", bufs=2,
                                       name=f"s{b}_{qs_i}_{ktc}")
                        nc.tensor.matmul(
                            s_ps[:, 0:512],
                            kt[0:64, ktc * 128:(ktc + 1) * 128],
                            qt[0:64, q0:q0 + 512],
                            start=True, stop=True, tile_position=(0, 0))
                        nc.tensor.matmul(
                            s_ps[:, 512:1024],
                            kt[64:128, ktc * 128:(ktc + 1) * 128],
                            qt[64:128, q0:q0 + 512],
                            start=True, stop=True, tile_position=(64, 0))
                        pt = sb.tile([128, 1024], BF16, tag="p_sb",
                                     bufs=3, name=f"p{b}_{qs_i}_{ktc}")
                        nc.scalar.activation(pt[:], s_ps[:], AF.Exp,
                                             scale=0.125)
                        nc.tensor.matmul(
                            pav[:, 0:512], vext[(b, ktc, 0)][:],
                            pt[:, 0:512],
                            start=(ktc == 0), stop=(ktc == NKT - 1))
                        nc.tensor.matmul(
                            pav[:, 512:1024], vext[(b, ktc, 1)][:],
                            pt[:, 512:1024],
                            start=(ktc == 0), stop=(ktc == NKT - 1))
                    # pav rows 0:64 = denominators, rows 64:128 = AV
                    # (cols 0:512 = h0, cols 512:1024 = h1)
                    araw = sb.tile([128, 1024], F32, tag="araw", bufs=2,
                                   name=f"ar{b}_{qs_i}")
                    nc.vector.tensor_copy(araw[:], pav[:])
                    rcf = sb.tile([128, 1024], F32, tag="rcf", bufs=2,
                                  name=f"rcf{b}_{qs_i}")
                    nc.vector.reciprocal_approx_fast(rcf[:], araw[:])
                    bcs = sb.tile([128, 1024], F32, tag="bcs", bufs=2,
                                  name=f"bcs{b}_{qs_i}")
                    nc.gpsimd.partition_broadcast(bcs[:], rcf[0:1, :])
                    nc.vector.tensor_mul(at0[64:128, q0:q0 + QS],
                                         araw[64:128, 0:512],
                                         bcs[64:128, 0:512])
                    nc.vector.tensor_mul(at1[64:128, q0:q0 + QS],
                                         araw[64:128, 512:1024],
                                         bcs[64:128, 512:1024])
                    # rows r<64 of a shard = h1 (head 2c+1), r>=64 = h0
                    if b < B - 1:
                        hf = qs_i // 2
                        for jj in range(4):
                            j = (qs_i % 2) * 4 + jj
                            tok = q0 + jj * HP
                            nc.sync.dma_start(
                                a2a_in[(b, hf)][j * 128:j * 128 + 64, :],
                                at1[64:128, tok:tok + HP])
                            nc.sync.dma_start(
                                a2a_in[(b, hf)][j * 128 + 64:(j + 1) * 128, :],
                                at0[64:128, tok:tok + HP])
                        if qs_i % 2 == 1:
                            nc.gpsimd.collective_compute(
                                "AllToAll", mybir.AluOpType.bypass,
                                replica_groups=[list(range(N_CORES))],
                                ins=[a2a_in[(b, hf)][:]],
                                outs=[a2a_out[(b, hf)][:]],
                            )
                    else:
                        # batch 3: one quarter-A2A per stripe
                        for j in range(8):
                            tok = q0 + j * QP
                            nc.sync.dma_start(
                                a2a_in[(3, qs_i)][j * 128:j * 128 + 64, :],
                                at1[64:128, tok:tok + QP])
                            nc.sync.dma_start(
                                a2a_in[(3, qs_i)][j * 128 + 64:(j + 1) * 128, :],
                                at0[64:128, tok:tok + QP])
                        nc.gpsimd.collective_compute(
                            "AllToAll", mybir.AluOpType.bypass,
                            replica_groups=[list(range(N_CORES))],
                            ins=[a2a_in[(3, qs_i)][:]],
                            outs=[a2a_out[(3, qs_i)][:]],
                        )

                def emit_ast_loads(b, parts):
                    ast = ast_tiles.get(b)
                    if ast is None:
                        ast = [sb.tile([128, TPC], BF16, tag="ast", bufs=18,
                                       name=f"ast{b}_{k}") for k in range(8)]
                        ast_tiles[b] = ast
                    w = HP if b < B - 1 else QP
                    for hf in parts:
                        for k in range(8):
                            nc.sync.dma_start(
                                ast[k][:, hf * w:(hf + 1) * w],
                                a2a_out[(b, hf)][k * 128:(k + 1) * 128, :])

                def emit_p3_ogroup(b, o, hf=None):
                    ast = ast_tiles[b]
                    w = HP if b < B - 1 else QP
                    c0, nc_ = (0, TPC) if hf is None else (hf * w, w)
                    po = ps.tile([128, TPC], F32, tag="proj", bufs=2,
                                 name=f"po{b}_{o}_{hf}")
                    for k in range(8):
                        nc.tensor.matmul(
                            po[:, 0:nc_],
                            wo_s[:, k * D + o * 128:k * D + (o + 1) * 128],
                            ast[k][:, c0:c0 + nc_],
                            start=(k == 0), stop=(k == 7))
                    ot = sb.tile([128, TPC], F32, tag="ot", bufs=3,
                                 name=f"ot{b}_{o}_{hf}")
                    nc.vector.tensor_scalar_add(ot[:, 0:nc_], po[:, 0:nc_],
                                                bias_t["bo"][:, o:o + 1])
                    nc.sync.dma_start(
                        outT[o * 128:(o + 1) * 128,
                             b * TPC + c0:b * TPC + c0 + nc_],
                        ot[:, 0:nc_])

                # ---------------- batch 0 fast-start ----------------------
                emit_p1_loads(0, 0)
                emit_p1_loads(0, 1)
                for w in (1, 2, 0):
                    emit_p1_group(0, 0, w)
                emit_vext_chunk(0, 0)
                # big Wo DMA deferred so batch-0 x loads win the queues
                wo_s = sb.tile([128, 8 * D], BF16, tag="wo_s", name="wo_s")
                for q in range(4):
                    nc.sync.dma_start(wo_s[:, q * 2048:(q + 1) * 2048],
                                      wo[:, q * 2048:(q + 1) * 2048])

                def sched(b):
                    """jobs[qs][ktc] for stripes of batch b."""
                    jobs = {qs: {} for qs in range(NQS)}

                    def add(qs, ktc, fn):
                        jobs[qs].setdefault(ktc, []).append(fn)

                    nb = b + 1
                    if b == 0:
                        # finish batch 0's own projections inside stripes 0/1
                        add(0, 0, lambda: emit_p1_group(0, 1, 1))
                        add(0, 2, lambda: emit_p1_group(0, 1, 2))
                        add(0, 3, lambda: emit_vext_chunk(0, 1))
                        add(0, 4, lambda: emit_p1_loads(0, 2))
                        add(0, 6, lambda: emit_p1_group(0, 2, 1))
                        add(0, 7, lambda: emit_p1_group(0, 2, 2))
                        add(0, 8, lambda: emit_vext_chunk(0, 2))
                        add(0, 9, lambda: emit_p1_loads(0, 3))
                        add(0, 10, lambda: emit_p1_group(0, 3, 1))
                        add(0, 11, lambda: emit_p1_group(0, 3, 2))
                        add(0, 12, lambda: emit_vext_chunk(0, 3))
                        add(0, 14, lambda: emit_p1_group(0, 1, 0))
                        add(1, 1, lambda: emit_p1_group(0, 2, 0))
                        add(1, 3, lambda: emit_p1_group(0, 3, 0))
                        add(1, 6, lambda: emit_p1_loads(1, 0))
                        add(1, 9, lambda: emit_p1_loads(1, 1))
                        add(1, 12, lambda: emit_p1_group(1, 0, 0))
                        add(1, 15, lambda: emit_p1_group(1, 0, 1))
                        add(2, 1, lambda: emit_p1_group(1, 0, 2))
                        add(2, 4, lambda: emit_p1_group(1, 1, 0))
                        add(2, 7, lambda: emit_p1_group(1, 1, 1))
                        add(2, 10, lambda: emit_p1_group(1, 1, 2))
                        add(2, 13, lambda: emit_p1_loads(1, 2))
                        add(2, 15, lambda: emit_vext_chunk(1, 0))
                        add(3, 1, lambda: emit_p1_group(1, 2, 0))
                        add(3, 3, lambda: emit_p1_group(1, 2, 1))
                        add(3, 5, lambda: emit_p1_group(1, 2, 2))
                        add(3, 7, lambda: emit_p1_loads(1, 3))
                        add(3, 9, lambda: emit_p1_group(1, 3, 0))
                        add(3, 11, lambda: emit_p1_group(1, 3, 1))
                        add(3, 13, lambda: emit_p1_group(1, 3, 2))
                        add(3, 15, lambda: emit_vext_chunk(1, 1))
                        return jobs

                    # b >= 1: weave next batch's projections + prev batch's
                    # out-proj (both halves landed a full batch ago).
                    add(0, 0, lambda pb=b - 1: emit_ast_loads(pb, (0, 1)))
                    for i in range(4):
                        add(1, 1 + 4 * i,
                            lambda pb=b - 1, o=i: emit_p3_ogroup(pb, o))
                        add(2, 1 + 4 * i,
                            lambda pb=b - 1, o=4 + i: emit_p3_ogroup(pb, o))
                    if nb < B:
                        add(0, 1, lambda: emit_p1_loads(nb, 0))
                        add(0, 4, lambda: emit_p1_loads(nb, 1))
                        add(0, 7, lambda: emit_p1_group(nb, 0, 0))
                        add(0, 10, lambda: emit_p1_group(nb, 0, 1))
                        add(0, 13, lambda: emit_p1_group(nb, 0, 2))
                        add(1, 3, lambda: emit_p1_group(nb, 1, 0))
                        add(1, 7, lambda: emit_p1_group(nb, 1, 1))
                        add(1, 11, lambda: emit_p1_group(nb, 1, 2))
                        add(1, 14, lambda: emit_p1_loads(nb, 2))
                        add(2, 3, lambda: emit_p1_group(nb, 2, 0))
                        add(2, 7, lambda: emit_p1_group(nb, 2, 1))
                        add(2, 11, lambda: emit_p1_group(nb, 2, 2))
                        add(2, 14, lambda: emit_p1_loads(nb, 3))
                        add(2, 15, lambda: emit_vext_chunk(nb, 0))
                        add(3, 1, lambda: emit_p1_group(nb, 3, 0))
                        add(3, 3, lambda: emit_p1_group(nb, 3, 1))
                        add(3, 5, lambda: emit_p1_group(nb, 3, 2))
                        add(3, 8, lambda: emit_vext_chunk(nb, 1))
                        add(3, 11, lambda: emit_vext_chunk(nb, 2))
                        add(3, 14, lambda: emit_vext_chunk(nb, 3))
                    else:
                        # batch 3: quarter q's out-proj woven into stripe
                        # q+2 (a full stripe after its A2A fires)
                        for q in range(2):
                            add(q + 2, 0, lambda qq=q: emit_ast_loads(3, (qq,)))
                            for i in range(8):
                                add(q + 2, 2 + int(i * 1.7),
                                    lambda qq=q, o=i: emit_p3_ogroup(
                                        3, o, hf=qq))
                    return jobs

                for b in range(B):
                    jobs = sched(b)
                    for qs_i in range(NQS):
                        emit_p2_stripe(b, qs_i, jobs[qs_i])
                    if b == 0:
                        # b0 fast-start had no room in qs3 for these
                        emit_vext_chunk(1, 2)
                        emit_vext_chunk(1, 3)

                # ------------- tail: batch 3 quarters 2 and 3 -------------
                emit_ast_loads(3, (2,))
                for o in range(8):
                    emit_p3_ogroup(3, o, hf=2)
                emit_ast_loads(3, (3,))
                for o in range(8):
                    emit_p3_ogroup(3, o, hf=3)

    nc.compile()
    return nc


def _get_nc():
    if "nc" not in _CACHE:
        _CACHE["nc"] = _build()
    return _CACHE["nc"]


def _make_in_maps(x, Wq, bq, Wk, bk, Wv, bv, Wo, bo):
    import ml_dtypes
    bf16 = ml_dtypes.bfloat16

    x = np.asarray(x, dtype=np.float32)
    Wq, Wk, Wv, Wo = (np.asarray(w, dtype=np.float32) for w in (Wq, Wk, Wv, Wo))
    bq, bk, bv, bo = (np.asarray(v, dtype=np.float32) for v in (bq, bk, bv, bo))

    xT = np.ascontiguousarray(x.reshape(TOK, D).T.astype(bf16))

    def warr(W, cs):
        # [128, 8*128]: chunk k cols <- W[k*128:(k+1)*128, cs]
        return np.ascontiguousarray(
            W[:, cs].reshape(8, 128, CW).transpose(1, 0, 2).reshape(128, D)
            .astype(bf16))

    # A2A-gathered A'^T row 128*i + r: r < 64 -> head 2i+1, r >= 64 ->
    # head 2i, dim r % 64.
    perm = np.empty(D, dtype=np.int64)
    for i in range(8):
        for r in range(128):
            h = 2 * i + (1 if r < 64 else 0)
            perm[128 * i + r] = h * 64 + (r % 64)
    wo_p = Wo[perm]  # [1024, 1024]
    wo_host = np.ascontiguousarray(
        wo_p.reshape(8, 128, D).transpose(1, 0, 2).reshape(128, 8 * D)
        .astype(bf16))

    bo_host = np.ascontiguousarray(bo.reshape(8, 128).T)

    in_maps = []
    for c in range(N_CORES):
        cs = slice(c * CW, (c + 1) * CW)
        in_maps.append({
            "xT": xT,
            "wq": warr(Wq, cs),
            "wk": warr(Wk, cs),
            "wv": warr(Wv, cs),
            "wo": wo_host,
            "bq": np.ascontiguousarray(bq[cs].reshape(CW, 1)),
            "bk": np.ascontiguousarray(bk[cs].reshape(CW, 1)),
            "bv": np.ascontiguousarray(bv[cs].reshape(CW, 1)),
            "bo": bo_host,
            "ident": np.eye(128, dtype=np.float32),
        })
    return in_maps


def kernel(x, Wq, bq, Wk, bk, Wv, bv, Wo, bo):
    from concourse import bass_utils

    in_maps = _make_in_maps(x, Wq, bq, Wk, bk, Wv, bv, Wo, bo)
    nc = _get_nc()
    res = bass_utils.run_bass_kernel_spmd(nc, in_maps,
                                          core_ids=list(range(N_CORES)))
    _CACHE["last_results"] = res

    out = np.empty((B, S, D), dtype=np.float32)
    QP = HP // 2
    for c in range(N_CORES):
        r = res.results[c]["outT"]  # [1024 odim, 4*256 token slots]
        for b in range(B - 1):
            for hf in range(2):
                t0 = b * S + hf * (S // 2) + c * HP
                out.reshape(TOK, D)[t0:t0 + HP, :] = \
                    r[:, b * TPC + hf * HP:b * TPC + (hf + 1) * HP].T
        for qq in range(4):
            t0 = 3 * S + qq * QS + c * QP
            out.reshape(TOK, D)[t0:t0 + QP, :] = \
                r[:, 3 * TPC + qq * QP:3 * TPC + (qq + 1) * QP].T
    return out


# revision 18
# speedup vs baseline: 1.2030x; 1.0412x over previous
"""Multi-head self-attention (B=4, S=2048, D=1024, H=16, Hd=64) on 8 TRN2 cores.

Sharding: tensor-parallel over heads for QKV+attention (core c owns heads
2c, 2c+1), token-parallel for the output projection (core c owns tokens
[b*2048 + hf*1024 + c*128, +128) for each half hf), bridged by two small
AllToAlls per batch (256 KB/rank, fired at mid-batch and batch-end) so
communication always completes a full batch before its consumers run.

All-bf16 datapath: x and weights are converted to bf16 on the host and
DMA'd directly into SBUF (no on-device casts). Per (batch, 512-query
stripe):
  - S^T pair: both heads' score matmuls issued back-to-back with
    tile_position (0,0)/(64,0) -> concurrent on the PE array (each uses
    only 64 contraction rows), into one [128,1024] PSUM tile.
  - one exp on ACT over both heads' scores ([128,1024], scale=1/8 folded).
  - AV per head with ve = [ones(64) | V(64)] so the softmax denominator
    lands in PSUM rows 0:64 and the AV rows in 64:128 for BOTH heads:
    every DVE op and the single partition_broadcast stay base-aligned.
  - normalize on DVE into per-head A^T tiles (rows 64:128 used).
Out-proj: lhsT = full (row-permuted) Wo, rhs = AllToAll-gathered A'^T
chunks, woven into the next batch's attention stripes; batch 3's halves
are consumed separately so the tail only waits for the last 256 KB A2A.
"""
import numpy as np

B, S, D, H, HD = 4, 2048, 1024, 16, 64
N_CORES = 8
TOK = B * S            # 8192
HPC = H // N_CORES     # 2 heads per core
CW = HPC * HD          # 128 cols per core
QS = 512               # query stripe
NKT = S // 128         # 16 kt chunks per batch
NQS = S // QS          # 4 q stripes per batch
TPC = S // N_CORES     # 256 tokens per (core, batch)
HP = TPC // 2          # 128 tokens per (core, batch, half)

_CACHE = {}


def _build():
    import concourse.bacc as bacc
    import concourse.mybir as mybir
    import concourse.tile as tile

    F32 = mybir.dt.float32
    F32R = mybir.dt.float32r
    BF16 = mybir.dt.bfloat16
    AF = mybir.ActivationFunctionType

    nc = bacc.Bacc(trn_type="TRN2", target_bir_lowering=False, debug=False,
                   num_devices=N_CORES)

    xT = nc.dram_tensor("xT", [D, TOK], BF16, kind="ExternalInput")
    wq = nc.dram_tensor("wq", [128, D], BF16, kind="ExternalInput")
    wk = nc.dram_tensor("wk", [128, D], BF16, kind="ExternalInput")
    wv = nc.dram_tensor("wv", [128, D], BF16, kind="ExternalInput")
    wo = nc.dram_tensor("wo", [128, 8 * D], BF16, kind="ExternalInput")
    bq = nc.dram_tensor("bq", [CW, 1], F32, kind="ExternalInput")
    bk = nc.dram_tensor("bk", [CW, 1], F32, kind="ExternalInput")
    bv = nc.dram_tensor("bv", [CW, 1], F32, kind="ExternalInput")
    bo = nc.dram_tensor("bo", [128, 8], F32, kind="ExternalInput")
    ident = nc.dram_tensor("ident", [128, 128], F32R, kind="ExternalInput")
    outT = nc.dram_tensor("outT", [D, B * TPC], F32, kind="ExternalOutput")

    with tile.TileContext(nc) as tc:
        with tc.tile_pool(name="sb", bufs=1) as sb, \
             tc.tile_pool(name="dram", bufs=1, space="DRAM") as dram:
            # ---------------- prologue: small weights, biases, constants --
            wq_s = sb.tile([128, D], BF16, tag="wq_s", name="wq_s")
            nc.sync.dma_start(wq_s[:], wq[:])
            wk_s = sb.tile([128, D], BF16, tag="wk_s", name="wk_s")
            nc.sync.dma_start(wk_s[:], wk[:])
            wv_s = sb.tile([128, D], BF16, tag="wv_s", name="wv_s")
            nc.sync.dma_start(wv_s[:], wv[:])
            identr = sb.tile([128, 128], F32R, tag="identr", name="identr")
            nc.sync.dma_start(identr[:], ident[:])

            bias_t = {}
            for bname, bdram, bshape in (("bq", bq, [CW, 1]),
                                         ("bk", bk, [CW, 1]),
                                         ("bv", bv, [CW, 1]),
                                         ("bo", bo, [128, 8])):
                bt_ = sb.tile(bshape, F32, tag=f"{bname}_t", name=f"{bname}_t")
                nc.sync.dma_start(bt_[:], bdram[:])
                bias_t[bname] = bt_

            a2a_in = {}
            a2a_out = {}
            for b in range(B - 1):
                for hf in range(2):
                    a2a_in[(b, hf)] = dram.tile(
                        [N_CORES * 128, HP], BF16,
                        tag=f"a2ai{b}{hf}", name=f"a2ai{b}{hf}")
                    a2a_out[(b, hf)] = dram.tile(
                        [N_CORES * 128, HP], BF16,
                        tag=f"a2ao{b}{hf}", name=f"a2ao{b}{hf}")
            # batch 3 split 3:1 -- part 0 = 192 tokens/core (fires after
            # qs2), part 1 = 64 tokens/core (fires at batch end)
            P3W = (192, 64)
            for hf in range(2):
                a2a_in[(3, hf)] = dram.tile(
                    [N_CORES * 128, P3W[hf]], BF16,
                    tag=f"a2ai3{hf}", name=f"a2ai3{hf}")
                a2a_out[(3, hf)] = dram.tile(
                    [N_CORES * 128, P3W[hf]], BF16,
                    tag=f"a2ao3{hf}", name=f"a2ao3{hf}")

            with tc.tile_pool(name="ps12", bufs=1, space="PSUM") as ps:
                qkv = {}
                xr_tiles = {}
                vext = {}
                at_tiles = {}
                ast_tiles = {}

                def emit_p1_loads(b, tb):
                    if tb == 0:
                        qkv[b] = (
                            sb.tile([128, S], BF16, tag="qt_sb", bufs=2,
                                    name=f"qt{b}"),
                            sb.tile([128, S], BF16, tag="kt_sb", bufs=2,
                                    name=f"kt{b}"),
                            sb.tile([128, S], F32R, tag="vt_sb", bufs=2,
                                    name=f"vt{b}"),
                        )
                        at_tiles[b] = (
                            sb.tile([128, S], BF16, tag="at0",
                                    bufs=2, name=f"at0_{b}"),
                            sb.tile([128, S], BF16, tag="at1",
                                    bufs=2, name=f"at1_{b}"),
                        )
                    g0 = b * S + tb * 512
                    xr = []
                    for k in range(8):
                        xk = sb.tile([128, 512], BF16, tag="xr", bufs=34,
                                     name=f"xr{b}_{tb}_{k}")
                        nc.sync.dma_start(
                            xk[:], xT[k * 128:(k + 1) * 128, g0:g0 + 512])
                        xr.append(xk)
                    xr_tiles[(b, tb)] = xr

                def emit_p1_group(b, tb, which):
                    qt, kt, vt = qkv[b]
                    xr = xr_tiles[(b, tb)]
                    w_, out_sb, bias = (
                        (wq_s, qt, bias_t["bq"]),
                        (wk_s, kt, bias_t["bk"]),
                        (wv_s, vt, bias_t["bv"]))[which]
                    pp = ps.tile([128, 512], F32, tag="proj", bufs=2,
                                 name=f"pp{b}_{tb}_{which}")
                    for k in range(8):
                        nc.tensor.matmul(
                            pp[:], w_[:, k * 128:(k + 1) * 128],
                            xr[k][:], start=(k == 0), stop=(k == 7))
                    nc.vector.tensor_scalar_add(
                        out_sb[:, tb * 512:(tb + 1) * 512], pp[:], bias[:])

                def emit_vext_chunk(b, tbi):
                    vt = qkv[b][2]
                    for ktc in range(4 * tbi, 4 * tbi + 4):
                        tp = ps.tile([128, 128], F32R, tag="proj", bufs=2,
                                     name=f"tp{b}_{ktc}")
                        nc.tensor.transpose(
                            tp[:], vt[:, ktc * 128:(ktc + 1) * 128],
                            identr[:])
                        # both heads: [ones | V] -> denom rows 0:64,
                        # AV rows 64:128 (everything stays base-aligned)
                        ve = sb.tile([128, 128], BF16, tag="vext",
                                     bufs=36, name=f"ve{b}_{ktc}")
                        nc.gpsimd.memset(ve[:, 0:64], 1.0)
                        nc.vector.tensor_copy(ve[:, 64:128], tp[:, 0:64])
                        ve2 = sb.tile([128, 128], BF16, tag="vext",
                                      bufs=36, name=f"v2{b}_{ktc}")
                        nc.gpsimd.memset(ve2[:, 0:64], 1.0)
                        nc.vector.tensor_copy(ve2[:, 64:128], tp[:, 64:128])
                        vext[(b, ktc, 0)] = ve
                        vext[(b, ktc, 1)] = ve2

                def emit_p2_stripe(b, qs_i, jobs):
                    qt, kt, vt = qkv[b]
                    at0, at1 = at_tiles[b]
                    q0 = qs_i * QS
                    pav = ps.tile([128, 1024], F32, tag="av", bufs=1,
                                  name=f"pav{b}_{qs_i}")
                    for ktc in range(NKT):
                        for job in jobs.get(ktc, ()):
                            job()
                        s_ps = ps.tile([128, 1024], F32, tag="s", bufs=2,
                                       name=f"s{b}_{qs_i}_{ktc}")
                        nc.tensor.matmul(
                            s_ps[:, 0:512],
                            kt[0:64, ktc * 128:(ktc + 1) * 128],
                            qt[0:64, q0:q0 + 512],
                            start=True, stop=True, tile_position=(0, 0))
                        nc.tensor.matmul(
                            s_ps[:, 512:1024],
                            kt[64:128, ktc * 128:(ktc + 1) * 128],
                            qt[64:128, q0:q0 + 512],
                            start=True, stop=True, tile_position=(64, 0))
                        pt = sb.tile([128, 1024], BF16, tag="p_sb",
                                     bufs=3, name=f"p{b}_{qs_i}_{ktc}")
                        nc.scalar.activation(pt[:], s_ps[:], AF.Exp,
                                             scale=0.125)
                        nc.tensor.matmul(
                            pav[:, 0:512], vext[(b, ktc, 0)][:],
                            pt[:, 0:512],
                            start=(ktc == 0), stop=(ktc == NKT - 1))
                        nc.tensor.matmul(
                            pav[:, 512:1024], vext[(b, ktc, 1)][:],
                            pt[:, 512:1024],
                            start=(ktc == 0), stop=(ktc == NKT - 1))
                    # pav rows 0:64 = denominators, rows 64:128 = AV
                    # (cols 0:512 = h0, cols 512:1024 = h1)
                    araw = sb.tile([128, 1024], F32, tag="araw", bufs=2,
                                   name=f"ar{b}_{qs_i}")
                    nc.vector.tensor_copy(araw[:], pav[:])
                    rcf = sb.tile([128, 1024], F32, tag="rcf", bufs=2,
                                  name=f"rcf{b}_{qs_i}")
                    nc.vector.reciprocal_approx_fast(rcf[:], araw[:])
                    bcs = sb.tile([128, 1024], F32, tag="bcs", bufs=2,
                                  name=f"bcs{b}_{qs_i}")
                    nc.gpsimd.partition_broadcast(bcs[:], rcf[0:1, :])
                    nc.vector.tensor_mul(at0[64:128, q0:q0 + QS],
                                         araw[64:128, 0:512],
                                         bcs[64:128, 0:512])
                    nc.vector.tensor_mul(at1[64:128, q0:q0 + QS],
                                         araw[64:128, 512:1024],
                                         bcs[64:128, 512:1024])
                    # rows r<64 of a shard = h1 (head 2c+1), r>=64 = h0
                    if b < B - 1:
                        hf = qs_i // 2
                        for jj in range(4):
                            j = (qs_i % 2) * 4 + jj
                            tok = q0 + jj * HP
                            nc.sync.dma_start(
                                a2a_in[(b, hf)][j * 128:j * 128 + 64, :],
                                at1[64:128, tok:tok + HP])
                            nc.sync.dma_start(
                                a2a_in[(b, hf)][j * 128 + 64:(j + 1) * 128, :],
                                at0[64:128, tok:tok + HP])
                        if qs_i % 2 == 1:
                            nc.gpsimd.collective_compute(
                                "AllToAll", mybir.AluOpType.bypass,
                                replica_groups=[list(range(N_CORES))],
                                ins=[a2a_in[(b, hf)][:]],
                                outs=[a2a_out[(b, hf)][:]],
                            )
                    elif qs_i >= 2:
                        # at rows cover tokens 0:1536 after qs2, rest
                        # after qs3; DMA straight from the at tiles
                        hf = qs_i - 2
                        w = (192, 64)[hf]
                        base = (0, 1536)[hf]
                        for j in range(8):
                            tok = base + j * w
                            nc.sync.dma_start(
                                a2a_in[(3, hf)][j * 128:j * 128 + 64, :],
                                at1[64:128, tok:tok + w])
                            nc.sync.dma_start(
                                a2a_in[(3, hf)][j * 128 + 64:(j + 1) * 128, :],
                                at0[64:128, tok:tok + w])
                        nc.gpsimd.collective_compute(
                            "AllToAll", mybir.AluOpType.bypass,
                            replica_groups=[list(range(N_CORES))],
                            ins=[a2a_in[(3, hf)][:]],
                            outs=[a2a_out[(3, hf)][:]],
                        )

                def emit_ast_loads(b, parts):
                    ast = ast_tiles.get(b)
                    if ast is None:
                        ast = [sb.tile([128, TPC], BF16, tag="ast", bufs=18,
                                       name=f"ast{b}_{k}") for k in range(8)]
                        ast_tiles[b] = ast
                    for hf in parts:
                        c0, w = ((0, HP) if hf == 0 else (HP, HP)) \
                            if b < B - 1 else ((0, 192) if hf == 0
                                               else (192, 64))
                        for k in range(8):
                            nc.sync.dma_start(
                                ast[k][:, c0:c0 + w],
                                a2a_out[(b, hf)][k * 128:(k + 1) * 128, :])

                def emit_p3_ogroup(b, o, hf=None):
                    ast = ast_tiles[b]
                    if hf is None:
                        c0, nc_ = 0, TPC
                    elif b < B - 1:
                        c0, nc_ = hf * HP, HP
                    else:
                        c0, nc_ = (0, 192) if hf == 0 else (192, 64)
                    po = ps.tile([128, TPC], F32, tag="proj", bufs=2,
                                 name=f"po{b}_{o}_{hf}")
                    for k in range(8):
                        nc.tensor.matmul(
                            po[:, 0:nc_],
                            wo_s[:, k * D + o * 128:k * D + (o + 1) * 128],
                            ast[k][:, c0:c0 + nc_],
                            start=(k == 0), stop=(k == 7))
                    ot = sb.tile([128, TPC], F32, tag="ot", bufs=3,
                                 name=f"ot{b}_{o}_{hf}")
                    nc.vector.tensor_scalar_add(ot[:, 0:nc_], po[:, 0:nc_],
                                                bias_t["bo"][:, o:o + 1])
                    nc.sync.dma_start(
                        outT[o * 128:(o + 1) * 128,
                             b * TPC + c0:b * TPC + c0 + nc_],
                        ot[:, 0:nc_])

                # ---------------- batch 0 fast-start ----------------------
                emit_p1_loads(0, 0)
                emit_p1_loads(0, 1)
                for w in (1, 2, 0):
                    emit_p1_group(0, 0, w)
                emit_vext_chunk(0, 0)
                # big Wo DMA deferred so batch-0 x loads win the queues
                wo_s = sb.tile([128, 8 * D], BF16, tag="wo_s", name="wo_s")
                for q in range(4):
                    nc.sync.dma_start(wo_s[:, q * 2048:(q + 1) * 2048],
                                      wo[:, q * 2048:(q + 1) * 2048])

                def sched(b):
                    """jobs[qs][ktc] for stripes of batch b."""
                    jobs = {qs: {} for qs in range(NQS)}

                    def add(qs, ktc, fn):
                        jobs[qs].setdefault(ktc, []).append(fn)

                    nb = b + 1
                    if b == 0:
                        # finish batch 0's own projections inside stripes 0/1
                        add(0, 0, lambda: emit_p1_group(0, 1, 1))
                        add(0, 2, lambda: emit_p1_group(0, 1, 2))
                        add(0, 3, lambda: emit_vext_chunk(0, 1))
                        add(0, 4, lambda: emit_p1_loads(0, 2))
                        add(0, 6, lambda: emit_p1_group(0, 2, 1))
                        add(0, 7, lambda: emit_p1_group(0, 2, 2))
                        add(0, 8, lambda: emit_vext_chunk(0, 2))
                        add(0, 9, lambda: emit_p1_loads(0, 3))
                        add(0, 10, lambda: emit_p1_group(0, 3, 1))
                        add(0, 11, lambda: emit_p1_group(0, 3, 2))
                        add(0, 12, lambda: emit_vext_chunk(0, 3))
                        add(0, 14, lambda: emit_p1_group(0, 1, 0))
                        add(1, 1, lambda: emit_p1_group(0, 2, 0))
                        add(1, 3, lambda: emit_p1_group(0, 3, 0))
                        add(1, 6, lambda: emit_p1_loads(1, 0))
                        add(1, 9, lambda: emit_p1_loads(1, 1))
                        add(1, 12, lambda: emit_p1_group(1, 0, 0))
                        add(1, 15, lambda: emit_p1_group(1, 0, 1))
                        add(2, 1, lambda: emit_p1_group(1, 0, 2))
                        add(2, 4, lambda: emit_p1_group(1, 1, 0))
                        add(2, 7, lambda: emit_p1_group(1, 1, 1))
                        add(2, 10, lambda: emit_p1_group(1, 1, 2))
                        add(2, 13, lambda: emit_p1_loads(1, 2))
                        add(2, 15, lambda: emit_vext_chunk(1, 0))
                        add(3, 1, lambda: emit_p1_group(1, 2, 0))
                        add(3, 3, lambda: emit_p1_group(1, 2, 1))
                        add(3, 5, lambda: emit_p1_group(1, 2, 2))
                        add(3, 7, lambda: emit_p1_loads(1, 3))
                        add(3, 9, lambda: emit_p1_group(1, 3, 0))
                        add(3, 11, lambda: emit_p1_group(1, 3, 1))
                        add(3, 13, lambda: emit_p1_group(1, 3, 2))
                        add(3, 15, lambda: emit_vext_chunk(1, 1))
                        return jobs

                    # b >= 1: weave next batch's projections + prev batch's
                    # out-proj (both halves landed a full batch ago).
                    add(0, 0, lambda pb=b - 1: emit_ast_loads(pb, (0, 1)))
                    for i in range(4):
                        add(1, 1 + 4 * i,
                            lambda pb=b - 1, o=i: emit_p3_ogroup(pb, o))
                        add(2, 1 + 4 * i,
                            lambda pb=b - 1, o=4 + i: emit_p3_ogroup(pb, o))
                    if nb < B:
                        add(0, 1, lambda: emit_p1_loads(nb, 0))
                        add(0, 4, lambda: emit_p1_loads(nb, 1))
                        add(0, 7, lambda: emit_p1_group(nb, 0, 0))
                        add(0, 10, lambda: emit_p1_group(nb, 0, 1))
                        add(0, 13, lambda: emit_p1_group(nb, 0, 2))
                        add(1, 3, lambda: emit_p1_group(nb, 1, 0))
                        add(1, 7, lambda: emit_p1_group(nb, 1, 1))
                        add(1, 11, lambda: emit_p1_group(nb, 1, 2))
                        add(1, 14, lambda: emit_p1_loads(nb, 2))
                        add(2, 3, lambda: emit_p1_group(nb, 2, 0))
                        add(2, 7, lambda: emit_p1_group(nb, 2, 1))
                        add(2, 11, lambda: emit_p1_group(nb, 2, 2))
                        add(2, 14, lambda: emit_p1_loads(nb, 3))
                        add(2, 15, lambda: emit_vext_chunk(nb, 0))
                        add(3, 1, lambda: emit_p1_group(nb, 3, 0))
                        add(3, 3, lambda: emit_p1_group(nb, 3, 1))
                        add(3, 5, lambda: emit_p1_group(nb, 3, 2))
                        add(3, 8, lambda: emit_vext_chunk(nb, 1))
                    return jobs

                for b in range(B):
                    jobs = sched(b)
                    for qs_i in range(NQS):
                        emit_p2_stripe(b, qs_i, jobs[qs_i])
                    if b + 1 < B:
                        emit_vext_chunk(b + 1, 2)
                        emit_vext_chunk(b + 1, 3)

                # ------------- tail: part-0 out-proj overlaps the tiny ----
                # ------------- part-1 A2A, then part-1 finishes -----------
                emit_ast_loads(3, (0,))
                for o in range(8):
                    emit_p3_ogroup(3, o, hf=0)
                emit_ast_loads(3, (1,))
                for o in range(8):
                    emit_p3_ogroup(3, o, hf=1)

    nc.compile()
    return nc


def _get_nc():
    if "nc" not in _CACHE:
        _CACHE["nc"] = _build()
    return _CACHE["nc"]


def _make_in_maps(x, Wq, bq, Wk, bk, Wv, bv, Wo, bo):
    import ml_dtypes
    bf16 = ml_dtypes.bfloat16

    x = np.asarray(x, dtype=np.float32)
    Wq, Wk, Wv, Wo = (np.asarray(w, dtype=np.float32) for w in (Wq, Wk, Wv, Wo))
    bq, bk, bv, bo = (np.asarray(v, dtype=np.float32) for v in (bq, bk, bv, bo))

    xT = np.ascontiguousarray(x.reshape(TOK, D).T.astype(bf16))

    def warr(W, cs):
        # [128, 8*128]: chunk k cols <- W[k*128:(k+1)*128, cs]
        return np.ascontiguousarray(
            W[:, cs].reshape(8, 128, CW).transpose(1, 0, 2).reshape(128, D)
            .astype(bf16))

    # A2A-gathered A'^T row 128*i + r: r < 64 -> head 2i+1, r >= 64 ->
    # head 2i, dim r % 64.
    perm = np.empty(D, dtype=np.int64)
    for i in range(8):
        for r in range(128):
            h = 2 * i + (1 if r < 64 else 0)
            perm[128 * i + r] = h * 64 + (r % 64)
    wo_p = Wo[perm]  # [1024, 1024]
    wo_host = np.ascontiguousarray(
        wo_p.reshape(8, 128, D).transpose(1, 0, 2).reshape(128, 8 * D)
        .astype(bf16))

    bo_host = np.ascontiguousarray(bo.reshape(8, 128).T)

    in_maps = []
    for c in range(N_CORES):
        cs = slice(c * CW, (c + 1) * CW)
        in_maps.append({
            "xT": xT,
            "wq": warr(Wq, cs),
            "wk": warr(Wk, cs),
            "wv": warr(Wv, cs),
            "wo": wo_host,
            "bq": np.ascontiguousarray(bq[cs].reshape(CW, 1)),
            "bk": np.ascontiguousarray(bk[cs].reshape(CW, 1)),
            "bv": np.ascontiguousarray(bv[cs].reshape(CW, 1)),
            "bo": bo_host,
            "ident": np.eye(128, dtype=np.float32),
        })
    return in_maps


def kernel(x, Wq, bq, Wk, bk, Wv, bv, Wo, bo):
    from concourse import bass_utils

    in_maps = _make_in_maps(x, Wq, bq, Wk, bk, Wv, bv, Wo, bo)
    nc = _get_nc()
    res = bass_utils.run_bass_kernel_spmd(nc, in_maps,
                                          core_ids=list(range(N_CORES)))
    _CACHE["last_results"] = res

    out = np.empty((B, S, D), dtype=np.float32)
    for c in range(N_CORES):
        r = res.results[c]["outT"]  # [1024 odim, 4*256 token slots]
        for b in range(B - 1):
            for hf in range(2):
                t0 = b * S + hf * (S // 2) + c * HP
                out.reshape(TOK, D)[t0:t0 + HP, :] = \
                    r[:, b * TPC + hf * HP:b * TPC + (hf + 1) * HP].T
        t0 = 3 * S + c * 192
        out.reshape(TOK, D)[t0:t0 + 192, :] = \
            r[:, 3 * TPC:3 * TPC + 192].T
        t0 = 3 * S + 1536 + c * 64
        out.reshape(TOK, D)[t0:t0 + 64, :] = \
            r[:, 3 * TPC + 192:4 * TPC].T
    return out


# revision 19
# speedup vs baseline: 1.2382x; 1.0293x over previous
"""Multi-head self-attention (B=4, S=2048, D=1024, H=16, Hd=64) on 8 TRN2 cores.

Sharding: tensor-parallel over heads for QKV+attention (core c owns heads
2c, 2c+1), token-parallel for the output projection (core c owns tokens
[b*2048 + hf*1024 + c*128, +128) for each half hf), bridged by two small
AllToAlls per batch (256 KB/rank, fired at mid-batch and batch-end) so
communication always completes a full batch before its consumers run.

All-bf16 datapath: x and weights are converted to bf16 on the host and
DMA'd directly into SBUF (no on-device casts). Per (batch, 512-query
stripe):
  - S^T pair: both heads' score matmuls issued back-to-back with
    tile_position (0,0)/(64,0) -> concurrent on the PE array (each uses
    only 64 contraction rows), into one [128,1024] PSUM tile.
  - one exp on ACT over both heads' scores ([128,1024], scale=1/8 folded).
  - AV per head with ve = [ones(64) | V(64)] so the softmax denominator
    lands in PSUM rows 0:64 and the AV rows in 64:128 for BOTH heads:
    every DVE op and the single partition_broadcast stay base-aligned.
  - normalize on DVE into per-head A^T tiles (rows 64:128 used).
Out-proj: lhsT = full (row-permuted) Wo, rhs = AllToAll-gathered A'^T
chunks, woven into the next batch's attention stripes; batch 3's halves
are consumed separately so the tail only waits for the last 256 KB A2A.
"""
import numpy as np

B, S, D, H, HD = 4, 2048, 1024, 16, 64
N_CORES = 8
TOK = B * S            # 8192
HPC = H // N_CORES     # 2 heads per core
CW = HPC * HD          # 128 cols per core
QS = 512               # query stripe
NKT = S // 128         # 16 kt chunks per batch
NQS = S // QS          # 4 q stripes per batch
TPC = S // N_CORES     # 256 tokens per (core, batch)
HP = TPC // 2          # 128 tokens per (core, batch, half)

_CACHE = {}


def _build():
    import concourse.bacc as bacc
    import concourse.mybir as mybir
    import concourse.tile as tile

    F32 = mybir.dt.float32
    F32R = mybir.dt.float32r
    BF16 = mybir.dt.bfloat16
    AF = mybir.ActivationFunctionType

    nc = bacc.Bacc(trn_type="TRN2", target_bir_lowering=False, debug=False,
                   num_devices=N_CORES)

    xT = nc.dram_tensor("xT", [D, TOK], BF16, kind="ExternalInput")
    wq = nc.dram_tensor("wq", [128, D], BF16, kind="ExternalInput")
    wk = nc.dram_tensor("wk", [128, D], BF16, kind="ExternalInput")
    wv = nc.dram_tensor("wv", [128, D], BF16, kind="ExternalInput")
    wo = nc.dram_tensor("wo", [128, 8 * D], BF16, kind="ExternalInput")
    bq = nc.dram_tensor("bq", [CW, 1], F32, kind="ExternalInput")
    bk = nc.dram_tensor("bk", [CW, 1], F32, kind="ExternalInput")
    bv = nc.dram_tensor("bv", [CW, 1], F32, kind="ExternalInput")
    bo = nc.dram_tensor("bo", [128, 8], F32, kind="ExternalInput")
    ident = nc.dram_tensor("ident", [128, 128], F32R, kind="ExternalInput")
    outT = nc.dram_tensor("outT", [D, B * TPC], F32, kind="ExternalOutput")

    with tile.TileContext(nc) as tc:
        with tc.tile_pool(name="sb", bufs=1) as sb, \
             tc.tile_pool(name="dram", bufs=1, space="DRAM") as dram:
            # ---------------- prologue: small weights, biases, constants --
            wq_s = sb.tile([128, D], BF16, tag="wq_s", name="wq_s")
            nc.sync.dma_start(wq_s[:], wq[:])
            wk_s = sb.tile([128, D], BF16, tag="wk_s", name="wk_s")
            nc.sync.dma_start(wk_s[:], wk[:])
            wv_s = sb.tile([128, D], BF16, tag="wv_s", name="wv_s")
            nc.sync.dma_start(wv_s[:], wv[:])
            identr = sb.tile([128, 128], F32R, tag="identr", name="identr")
            nc.sync.dma_start(identr[:], ident[:])

            bias_t = {}
            for bname, bdram, bshape in (("bq", bq, [CW, 1]),
                                         ("bk", bk, [CW, 1]),
                                         ("bv", bv, [CW, 1]),
                                         ("bo", bo, [128, 8])):
                bt_ = sb.tile(bshape, F32, tag=f"{bname}_t", name=f"{bname}_t")
                nc.sync.dma_start(bt_[:], bdram[:])
                bias_t[bname] = bt_

            a2a_in = {}
            a2a_out = {}
            for b in range(B - 1):
                for hf in range(2):
                    a2a_in[(b, hf)] = dram.tile(
                        [N_CORES * 128, HP], BF16,
                        tag=f"a2ai{b}{hf}", name=f"a2ai{b}{hf}")
                    a2a_out[(b, hf)] = dram.tile(
                        [N_CORES * 128, HP], BF16,
                        tag=f"a2ao{b}{hf}", name=f"a2ao{b}{hf}")
            # batch 3 split 3:1 -- part 0 = 192 tokens/core (fires after
            # qs2), part 1 = 64 tokens/core (fires at batch end)
            P3W = (192, 64)
            for hf in range(2):
                a2a_in[(3, hf)] = dram.tile(
                    [N_CORES * 128, P3W[hf]], BF16,
                    tag=f"a2ai3{hf}", name=f"a2ai3{hf}")
                a2a_out[(3, hf)] = dram.tile(
                    [N_CORES * 128, P3W[hf]], BF16,
                    tag=f"a2ao3{hf}", name=f"a2ao3{hf}")

            with tc.tile_pool(name="ps12", bufs=1, space="PSUM") as ps:
                qkv = {}
                xr_tiles = {}
                vext = {}
                at_tiles = {}
                ast_tiles = {}

                def emit_p1_loads(b, tb):
                    if tb == 0:
                        qkv[b] = (
                            sb.tile([128, S], BF16, tag="qt_sb", bufs=2,
                                    name=f"qt{b}"),
                            sb.tile([128, S], BF16, tag="kt_sb", bufs=2,
                                    name=f"kt{b}"),
                            sb.tile([128, S], F32R, tag="vt_sb", bufs=2,
                                    name=f"vt{b}"),
                        )
                        at_tiles[b] = (
                            sb.tile([128, S], BF16, tag="at0",
                                    bufs=2, name=f"at0_{b}"),
                            sb.tile([128, S], BF16, tag="at1",
                                    bufs=2, name=f"at1_{b}"),
                        )
                    g0 = b * S + tb * 512
                    xr = []
                    for k in range(8):
                        xk = sb.tile([128, 512], BF16, tag="xr", bufs=34,
                                     name=f"xr{b}_{tb}_{k}")
                        nc.sync.dma_start(
                            xk[:], xT[k * 128:(k + 1) * 128, g0:g0 + 512])
                        xr.append(xk)
                    xr_tiles[(b, tb)] = xr

                def emit_p1_group(b, tb, which):
                    qt, kt, vt = qkv[b]
                    xr = xr_tiles[(b, tb)]
                    w_, out_sb, bias = (
                        (wq_s, qt, bias_t["bq"]),
                        (wk_s, kt, bias_t["bk"]),
                        (wv_s, vt, bias_t["bv"]))[which]
                    pp = ps.tile([128, 512], F32, tag="proj", bufs=2,
                                 name=f"pp{b}_{tb}_{which}")
                    for k in range(8):
                        nc.tensor.matmul(
                            pp[:], w_[:, k * 128:(k + 1) * 128],
                            xr[k][:], start=(k == 0), stop=(k == 7))
                    nc.vector.tensor_scalar_add(
                        out_sb[:, tb * 512:(tb + 1) * 512], pp[:], bias[:])

                def emit_vext_chunk(b, tbi):
                    vt = qkv[b][2]
                    for ktc in range(4 * tbi, 4 * tbi + 4):
                        tp = ps.tile([128, 128], F32R, tag="proj", bufs=2,
                                     name=f"tp{b}_{ktc}")
                        nc.tensor.transpose(
                            tp[:], vt[:, ktc * 128:(ktc + 1) * 128],
                            identr[:])
                        # both heads: [ones | V] -> denom rows 0:64,
                        # AV rows 64:128 (everything stays base-aligned)
                        ve = sb.tile([128, 128], BF16, tag="vext",
                                     bufs=36, name=f"ve{b}_{ktc}")
                        nc.gpsimd.memset(ve[:, 0:64], 1.0)
                        nc.vector.tensor_copy(ve[:, 64:128], tp[:, 0:64])
                        ve2 = sb.tile([128, 128], BF16, tag="vext",
                                      bufs=36, name=f"v2{b}_{ktc}")
                        nc.gpsimd.memset(ve2[:, 0:64], 1.0)
                        nc.vector.tensor_copy(ve2[:, 64:128], tp[:, 64:128])
                        vext[(b, ktc, 0)] = ve
                        vext[(b, ktc, 1)] = ve2

                def emit_p2_stripe(b, qs_i, jobs):
                    qt, kt, vt = qkv[b]
                    at0, at1 = at_tiles[b]
                    q0 = qs_i * QS
                    pav = ps.tile([128, 1024], F32, tag="av", bufs=1,
                                  name=f"pav{b}_{qs_i}")

                    def emit_av(ktc, pt):
                        nc.tensor.matmul(
                            pav[:, 0:512], vext[(b, ktc, 0)][:],
                            pt[:, 0:512],
                            start=(ktc == 0), stop=(ktc == NKT - 1))
                        nc.tensor.matmul(
                            pav[:, 512:1024], vext[(b, ktc, 1)][:],
                            pt[:, 512:1024],
                            start=(ktc == 0), stop=(ktc == NKT - 1))

                    pts = []
                    for ktc in range(NKT):
                        for job in jobs.get(ktc, ()):
                            job()
                        s_ps = ps.tile([128, 1024], F32, tag="s", bufs=2,
                                       name=f"s{b}_{qs_i}_{ktc}")
                        nc.tensor.matmul(
                            s_ps[:, 0:512],
                            kt[0:64, ktc * 128:(ktc + 1) * 128],
                            qt[0:64, q0:q0 + 512],
                            start=True, stop=True, tile_position=(0, 0))
                        nc.tensor.matmul(
                            s_ps[:, 512:1024],
                            kt[64:128, ktc * 128:(ktc + 1) * 128],
                            qt[64:128, q0:q0 + 512],
                            start=True, stop=True, tile_position=(64, 0))
                        pt = sb.tile([128, 1024], BF16, tag="p_sb",
                                     bufs=3, name=f"p{b}_{qs_i}_{ktc}")
                        nc.scalar.activation(pt[:], s_ps[:], AF.Exp,
                                             scale=0.125)
                        pts.append(pt)
                        # AV lags 2 chunks behind so its exp is already
                        # done when the PE FIFO reaches it
                        if ktc >= 2:
                            emit_av(ktc - 2, pts[ktc - 2])
                    emit_av(NKT - 2, pts[NKT - 2])
                    emit_av(NKT - 1, pts[NKT - 1])
                    # pav rows 0:64 = denominators, rows 64:128 = AV
                    # (cols 0:512 = h0, cols 512:1024 = h1)
                    araw = sb.tile([128, 1024], F32, tag="araw", bufs=2,
                                   name=f"ar{b}_{qs_i}")
                    nc.vector.tensor_copy(araw[:], pav[:])
                    rcf = sb.tile([128, 1024], F32, tag="rcf", bufs=2,
                                  name=f"rcf{b}_{qs_i}")
                    nc.vector.reciprocal_approx_fast(rcf[:], araw[:])
                    bcs = sb.tile([128, 1024], F32, tag="bcs", bufs=2,
                                  name=f"bcs{b}_{qs_i}")
                    nc.gpsimd.partition_broadcast(bcs[:], rcf[0:1, :])
                    nc.vector.tensor_mul(at0[64:128, q0:q0 + QS],
                                         araw[64:128, 0:512],
                                         bcs[64:128, 0:512])
                    nc.vector.tensor_mul(at1[64:128, q0:q0 + QS],
                                         araw[64:128, 512:1024],
                                         bcs[64:128, 512:1024])
                    # rows r<64 of a shard = h1 (head 2c+1), r>=64 = h0
                    if b < B - 1:
                        hf = qs_i // 2
                        for jj in range(4):
                            j = (qs_i % 2) * 4 + jj
                            tok = q0 + jj * HP
                            nc.sync.dma_start(
                                a2a_in[(b, hf)][j * 128:j * 128 + 64, :],
                                at1[64:128, tok:tok + HP])
                            nc.sync.dma_start(
                                a2a_in[(b, hf)][j * 128 + 64:(j + 1) * 128, :],
                                at0[64:128, tok:tok + HP])
                        if qs_i % 2 == 1:
                            nc.gpsimd.collective_compute(
                                "AllToAll", mybir.AluOpType.bypass,
                                replica_groups=[list(range(N_CORES))],
                                ins=[a2a_in[(b, hf)][:]],
                                outs=[a2a_out[(b, hf)][:]],
                            )
                    elif qs_i >= 2:
                        # at rows cover tokens 0:1536 after qs2, rest
                        # after qs3; DMA straight from the at tiles
                        hf = qs_i - 2
                        w = (192, 64)[hf]
                        base = (0, 1536)[hf]
                        for j in range(8):
                            tok = base + j * w
                            nc.sync.dma_start(
                                a2a_in[(3, hf)][j * 128:j * 128 + 64, :],
                                at1[64:128, tok:tok + w])
                            nc.sync.dma_start(
                                a2a_in[(3, hf)][j * 128 + 64:(j + 1) * 128, :],
                                at0[64:128, tok:tok + w])
                        nc.gpsimd.collective_compute(
                            "AllToAll", mybir.AluOpType.bypass,
                            replica_groups=[list(range(N_CORES))],
                            ins=[a2a_in[(3, hf)][:]],
                            outs=[a2a_out[(3, hf)][:]],
                        )

                def emit_ast_loads(b, parts):
                    ast = ast_tiles.get(b)
                    if ast is None:
                        ast = [sb.tile([128, TPC], BF16, tag="ast", bufs=18,
                                       name=f"ast{b}_{k}") for k in range(8)]
                        ast_tiles[b] = ast
                    for hf in parts:
                        c0, w = ((0, HP) if hf == 0 else (HP, HP)) \
                            if b < B - 1 else ((0, 192) if hf == 0
                                               else (192, 64))
                        for k in range(8):
                            nc.sync.dma_start(
                                ast[k][:, c0:c0 + w],
                                a2a_out[(b, hf)][k * 128:(k + 1) * 128, :])

                def emit_p3_ogroup(b, o, hf=None):
                    ast = ast_tiles[b]
                    if hf is None:
                        c0, nc_ = 0, TPC
                    elif b < B - 1:
                        c0, nc_ = hf * HP, HP
                    else:
                        c0, nc_ = (0, 192) if hf == 0 else (192, 64)
                    po = ps.tile([128, TPC], F32, tag="proj", bufs=2,
                                 name=f"po{b}_{o}_{hf}")
                    for k in range(8):
                        nc.tensor.matmul(
                            po[:, 0:nc_],
                            wo_s[:, k * D + o * 128:k * D + (o + 1) * 128],
                            ast[k][:, c0:c0 + nc_],
                            start=(k == 0), stop=(k == 7))
                    ot = sb.tile([128, TPC], F32, tag="ot", bufs=3,
                                 name=f"ot{b}_{o}_{hf}")
                    nc.vector.tensor_scalar_add(ot[:, 0:nc_], po[:, 0:nc_],
                                                bias_t["bo"][:, o:o + 1])
                    nc.sync.dma_start(
                        outT[o * 128:(o + 1) * 128,
                             b * TPC + c0:b * TPC + c0 + nc_],
                        ot[:, 0:nc_])

                # ---------------- batch 0 fast-start ----------------------
                emit_p1_loads(0, 0)
                emit_p1_loads(0, 1)
                for w in (1, 2, 0):
                    emit_p1_group(0, 0, w)
                emit_vext_chunk(0, 0)
                # big Wo DMA deferred so batch-0 x loads win the queues
                wo_s = sb.tile([128, 8 * D], BF16, tag="wo_s", name="wo_s")
                for q in range(4):
                    nc.sync.dma_start(wo_s[:, q * 2048:(q + 1) * 2048],
                                      wo[:, q * 2048:(q + 1) * 2048])

                def sched(b):
                    """jobs[qs][ktc] for stripes of batch b."""
                    jobs = {qs: {} for qs in range(NQS)}

                    def add(qs, ktc, fn):
                        jobs[qs].setdefault(ktc, []).append(fn)

                    nb = b + 1
                    if b == 0:
                        # finish batch 0's own projections inside stripes 0/1
                        add(0, 0, lambda: emit_p1_group(0, 1, 1))
                        add(0, 2, lambda: emit_p1_group(0, 1, 2))
                        add(0, 3, lambda: emit_vext_chunk(0, 1))
                        add(0, 4, lambda: emit_p1_loads(0, 2))
                        add(0, 6, lambda: emit_p1_group(0, 2, 1))
                        add(0, 7, lambda: emit_p1_group(0, 2, 2))
                        add(0, 8, lambda: emit_vext_chunk(0, 2))
                        add(0, 9, lambda: emit_p1_loads(0, 3))
                        add(0, 10, lambda: emit_p1_group(0, 3, 1))
                        add(0, 11, lambda: emit_p1_group(0, 3, 2))
                        add(0, 12, lambda: emit_vext_chunk(0, 3))
                        add(0, 14, lambda: emit_p1_group(0, 1, 0))
                        add(1, 1, lambda: emit_p1_group(0, 2, 0))
                        add(1, 3, lambda: emit_p1_group(0, 3, 0))
                        add(1, 6, lambda: emit_p1_loads(1, 0))
                        add(1, 9, lambda: emit_p1_loads(1, 1))
                        add(1, 12, lambda: emit_p1_group(1, 0, 0))
                        add(1, 15, lambda: emit_p1_group(1, 0, 1))
                        add(2, 1, lambda: emit_p1_group(1, 0, 2))
                        add(2, 4, lambda: emit_p1_group(1, 1, 0))
                        add(2, 7, lambda: emit_p1_group(1, 1, 1))
                        add(2, 10, lambda: emit_p1_group(1, 1, 2))
                        add(2, 13, lambda: emit_p1_loads(1, 2))
                        add(2, 15, lambda: emit_vext_chunk(1, 0))
                        add(3, 1, lambda: emit_p1_group(1, 2, 0))
                        add(3, 3, lambda: emit_p1_group(1, 2, 1))
                        add(3, 5, lambda: emit_p1_group(1, 2, 2))
                        add(3, 7, lambda: emit_p1_loads(1, 3))
                        add(3, 9, lambda: emit_p1_group(1, 3, 0))
                        add(3, 11, lambda: emit_p1_group(1, 3, 1))
                        add(3, 13, lambda: emit_p1_group(1, 3, 2))
                        add(3, 15, lambda: emit_vext_chunk(1, 1))
                        return jobs

                    # b >= 1: weave next batch's projections + prev batch's
                    # out-proj (both halves landed a full batch ago).
                    add(0, 0, lambda pb=b - 1: emit_ast_loads(pb, (0, 1)))
                    for i in range(4):
                        add(1, 1 + 4 * i,
                            lambda pb=b - 1, o=i: emit_p3_ogroup(pb, o))
                        add(2, 1 + 4 * i,
                            lambda pb=b - 1, o=4 + i: emit_p3_ogroup(pb, o))
                    if nb < B:
                        add(0, 1, lambda: emit_p1_loads(nb, 0))
                        add(0, 4, lambda: emit_p1_loads(nb, 1))
                        add(0, 7, lambda: emit_p1_group(nb, 0, 0))
                        add(0, 10, lambda: emit_p1_group(nb, 0, 1))
                        add(0, 13, lambda: emit_p1_group(nb, 0, 2))
                        add(1, 3, lambda: emit_p1_group(nb, 1, 0))
                        add(1, 7, lambda: emit_p1_group(nb, 1, 1))
                        add(1, 11, lambda: emit_p1_group(nb, 1, 2))
                        add(1, 14, lambda: emit_p1_loads(nb, 2))
                        add(2, 3, lambda: emit_p1_group(nb, 2, 0))
                        add(2, 7, lambda: emit_p1_group(nb, 2, 1))
                        add(2, 11, lambda: emit_p1_group(nb, 2, 2))
                        add(2, 14, lambda: emit_p1_loads(nb, 3))
                        add(2, 15, lambda: emit_vext_chunk(nb, 0))
                        add(3, 1, lambda: emit_p1_group(nb, 3, 0))
                        add(3, 3, lambda: emit_p1_group(nb, 3, 1))
                        add(3, 5, lambda: emit_p1_group(nb, 3, 2))
                        add(3, 8, lambda: emit_vext_chunk(nb, 1))
                        add(3, 11, lambda: emit_vext_chunk(nb, 2))
                        add(3, 14, lambda: emit_vext_chunk(nb, 3))
                    return jobs

                for b in range(B):
                    jobs = sched(b)
                    for qs_i in range(NQS):
                        emit_p2_stripe(b, qs_i, jobs[qs_i])
                    if b == 0:
                        # b0's fast-start schedule has no qs3 room for these
                        emit_vext_chunk(1, 2)
                        emit_vext_chunk(1, 3)

                # ------------- tail: part-0 out-proj overlaps the tiny ----
                # ------------- part-1 A2A, then part-1 finishes -----------
                emit_ast_loads(3, (0,))
                for o in range(8):
                    emit_p3_ogroup(3, o, hf=0)
                emit_ast_loads(3, (1,))
                for o in range(8):
                    emit_p3_ogroup(3, o, hf=1)

    nc.compile()
    return nc


def _get_nc():
    if "nc" not in _CACHE:
        _CACHE["nc"] = _build()
    return _CACHE["nc"]


def _make_in_maps(x, Wq, bq, Wk, bk, Wv, bv, Wo, bo):
    import ml_dtypes
    bf16 = ml_dtypes.bfloat16

    x = np.asarray(x, dtype=np.float32)
    Wq, Wk, Wv, Wo = (np.asarray(w, dtype=np.float32) for w in (Wq, Wk, Wv, Wo))
    bq, bk, bv, bo = (np.asarray(v, dtype=np.float32) for v in (bq, bk, bv, bo))

    xT = np.ascontiguousarray(x.reshape(TOK, D).T.astype(bf16))

    def warr(W, cs):
        # [128, 8*128]: chunk k cols <- W[k*128:(k+1)*128, cs]
        return np.ascontiguousarray(
            W[:, cs].reshape(8, 128, CW).transpose(1, 0, 2).reshape(128, D)
            .astype(bf16))

    # A2A-gathered A'^T row 128*i + r: r < 64 -> head 2i+1, r >= 64 ->
    # head 2i, dim r % 64.
    perm = np.empty(D, dtype=np.int64)
    for i in range(8):
        for r in range(128):
            h = 2 * i + (1 if r < 64 else 0)
            perm[128 * i + r] = h * 64 + (r % 64)
    wo_p = Wo[perm]  # [1024, 1024]
    wo_host = np.ascontiguousarray(
        wo_p.reshape(8, 128, D).transpose(1, 0, 2).reshape(128, 8 * D)
        .astype(bf16))

    bo_host = np.ascontiguousarray(bo.reshape(8, 128).T)

    in_maps = []
    for c in range(N_CORES):
        cs = slice(c * CW, (c + 1) * CW)
        in_maps.append({
            "xT": xT,
            "wq": warr(Wq, cs),
            "wk": warr(Wk, cs),
            "wv": warr(Wv, cs),
            "wo": wo_host,
            "bq": np.ascontiguousarray(bq[cs].reshape(CW, 1)),
            "bk": np.ascontiguousarray(bk[cs].reshape(CW, 1)),
            "bv": np.ascontiguousarray(bv[cs].reshape(CW, 1)),
            "bo": bo_host,
            "ident": np.eye(128, dtype=np.float32),
        })
    return in_maps


def kernel(x, Wq, bq, Wk, bk, Wv, bv, Wo, bo):
    from concourse import bass_utils

    in_maps = _make_in_maps(x, Wq, bq, Wk, bk, Wv, bv, Wo, bo)
    nc = _get_nc()
    res = bass_utils.run_bass_kernel_spmd(nc, in_maps,
                                          core_ids=list(range(N_CORES)))
    _CACHE["last_results"] = res

    out = np.empty((B, S, D), dtype=np.float32)
    for c in range(N_CORES):
        r = res.results[c]["outT"]  # [1024 odim, 4*256 token slots]
        for b in range(B - 1):
            for hf in range(2):
                t0 = b * S + hf * (S // 2) + c * HP
                out.reshape(TOK, D)[t0:t0 + HP, :] = \
                    r[:, b * TPC + hf * HP:b * TPC + (hf + 1) * HP].T
        t0 = 3 * S + c * 192
        out.reshape(TOK, D)[t0:t0 + 192, :] = \
            r[:, 3 * TPC:3 * TPC + 192].T
        t0 = 3 * S + 1536 + c * 64
        out.reshape(TOK, D)[t0:t0 + 64, :] = \
            r[:, 3 * TPC + 192:4 * TPC].T
    return out
